# revision 1
# baseline (speedup 1.0000x reference)
"""GCLSTM (ChebConv-gated LSTM) Trainium2 kernel, 8-core SPMD.

Algorithm notes
---------------
reference computes, per timestep t (T=24) over N=5120 graph nodes:
    gate_g = X_t @ Ws[g] + cheb(H, thetas[g]) + biases      (4 gates)
    cheb(H, th) = H@th0 + (L@H)@th1 + (2L(LH) - H)@th2      (K=3 Chebyshev)
with L the scaled-normalized graph Laplacian (5120x5120, sparse, here
densified).  The Chebyshev basis (U = L@H, V = L^2@H) is shared by all 4
gates, so per step we need exactly ONE dense "mega-prop" [U|V] = [L;L^2]@H
plus the gate matmuls.  Folding:
    gate_g = X_t@Ws[g] + H@(th0-th2) + U@th1 + V@(2*th2) + b
so all gate work is a single [X;H;U;V] (1024) x Theta (1024x1024) matmul.

Sharding: nodes are split across 8 cores (640 each; edges connect
arbitrary nodes, so each core holds the full [L;L^2] column block for its
output rows, resident in SBUF).  The mega-prop contracts over ALL 5120
nodes, so the full H (node-major) is re-assembled every step with two
feature-half AllGathers (~13us each on the critical path); everything
else stays core-local.

Precision: the mega-prop (74% of all PE cycles) runs in fp8-e4m3 with
DoubleRow perf mode (2 fp8 weights per PE cell -> 2 MACs/cycle):
  - [L;L^2] is static: host-quantized to fp8 with global scales SL/SL2.
  - H for the prop is quantized on-device to SH*H fp8 right before the
    AllGather (which therefore also moves half the bytes).  The prop
    PSUM result (SH*SL*U fp32) is rescaled back to exact U at the
    PSUM->SBUF copy, so the gate matmuls see unscaled fp16 U/V.
The gate matmuls stay fp16: X quantization is numerically catastrophic
(X dominates the gate pre-activations; fp8 X alone costs 1.3e-1 rel
err vs 6.8e-3 for the whole prop-fp8 scheme).

Schedule (from NTFF profile analysis): the step's critical chain is
AllGather -> prop -> UV-gates -> LSTM -> transpose/AG-prep.  To keep
the PE fed during the AllGather (and the HAM clock-gate warm), the
contraction is ordered own-nodes-first: ll2 holds [5 own tiles + 1 zero
pad + 40 global tiles with own rows zeroed], and the 3 own pairs per
output half multiply against the locally produced node-major H (hnm8,
no AllGather needed).  DMA queues are split (arrival DMAs on sync,
transposes/agin/x-loads on the scalar HWDGE queue), the fp8 converts
run on gpsimd right before the collectives they feed, and the gate
PSUM->gacc copies/adds run on gpsimd to unload the vector engine,
whose LSTM chain is otherwise on the critical path.
"""
import sys

for _p in ("/opt/trn_rl_repo",):
    if _p not in sys.path:
        sys.path.insert(0, _p)

import numpy as np
import concourse.bass as bass
import concourse.mybir as mybir
import concourse.tile as tile
from concourse import bacc
from concourse.bass_utils import run_bass_kernel_spmd

fp32 = mybir.dt.float32
fp16 = mybir.dt.float16
fp8 = mybir.dt.float8e4
np_fp8 = mybir.dt.np(fp8)

NCORES = 8
B, T, NTOW, F = 512, 24, 10, 256
N = B * NTOW                  # 5120 nodes
NLOC = N // NCORES            # 640 nodes per core
KT = N // 128                 # 40 contraction tiles over nodes
KOWN = 6                      # 5 own tiles + 1 zero pad (even for pairing)
KPO = KOWN // 2               # 3 own DoubleRow pairs
KPF = KT // 2                 # 20 foreign DoubleRow pairs
FT = F // 128                 # 2 feature tiles
GM = (4 * F) // 128           # 8 gate-feature m-tiles
NOUT2 = 2 * NLOC              # 1280 = [U|V] output columns per core
LAMBDA_MAX = 2.0

NCH = [(0, 512), (512, 640)]             # node chunks for gate matmuls
PCH = [(0, 512), (512, 1024), (1024, 1280)]  # [U|V] column chunks

SH = 128.0       # H -> fp8 scale for the prop (|H| < 1)
ATH = 1024.0     # theta U/V blocks -> fp8 (|2*th2| <= ~0.22 -> <=222)
SU = 32.0        # U -> fp8 (|U| <~ 3)
SV = 32.0        # V -> fp8 (|V| <~ 6)
G = ATH * SU     # common gate scale: X/H theta blocks (fp16) carry G,
assert G == ATH * SV  # undone for free by the LSTM activation scale

SIG = mybir.ActivationFunctionType.Sigmoid
TANH = mybir.ActivationFunctionType.Tanh
DR = mybir.MatmulPerfMode.DoubleRow

_CACHE = {}


def _build_nc(repeat=1, no_comm=False, split_ag=True):
    nc = bacc.Bacc(None, target_bir_lowering=False, num_devices=NCORES)
    d_ll2 = nc.dram_tensor("ll2", [KOWN + KT, 128, NOUT2], fp8,
                           kind="ExternalInput")
    d_th = nc.dram_tensor("th", [GM, 128, 4 * F], fp16, kind="ExternalInput")
    d_th8 = nc.dram_tensor("th8", [4, 128, 4 * F], fp8, kind="ExternalInput")
    d_x = nc.dram_tensor("xall", [T, FT, 128, NLOC], fp16, kind="ExternalInput")
    d_bias = nc.dram_tensor("biasv", [GM, 128], fp32, kind="ExternalInput")
    d_ruv = nc.dram_tensor("ruv", [2, 128], fp32, kind="ExternalInput")
    d_h = nc.dram_tensor("hout", [FT, 128, NLOC], fp32, kind="ExternalOutput")
    d_c = nc.dram_tensor("cout", [FT, 128, NLOC], fp32, kind="ExternalOutput")

    with tile.TileContext(nc) as tc:
        with (
            tc.tile_pool(name="const", bufs=1) as constp,
            tc.tile_pool(name="xp", bufs=2) as xp,
            tc.tile_pool(name="gp", bufs=2) as gp,
            tc.tile_pool(name="uvp", bufs=1) as uvp,
            tc.tile_pool(name="hp", bufs=2) as hp,
            tc.tile_pool(name="hnmp", bufs=2) as hnmp,
            tc.tile_pool(name="tmpp", bufs=1) as tmpp,
            tc.tile_pool(name="psg", bufs=4, space="PSUM") as psg,
            tc.tile_pool(name="psp", bufs=4, space="PSUM") as psp,
            tc.tile_pool(name="dramio", bufs=2, space="DRAM") as dramp,
        ):
            # ---- resident tensors ----
            sb_ll2 = constp.tile([128, KOWN + KT, NOUT2], fp8, tag="ll2")
            sb_th = constp.tile([128, GM, 4 * F], fp16, tag="th")
            sb_th8 = constp.tile([128, 4, 4 * F], fp8, tag="th8")
            nc.sync.dma_start(sb_th8, d_th8.rearrange("k p j -> p k j"))
            sb_bias = constp.tile([128, GM], fp32, tag="bias")
            sb_ruv = constp.tile([128, 2], fp32, tag="ruv")
            sb_hfull = constp.tile([128, KT, F], fp8, tag="hfull")
            nc.sync.dma_start(sb_bias, d_bias.rearrange("m p -> p m"))
            nc.sync.dma_start(sb_ruv, d_ruv.rearrange("m p -> p m"))
            # theta in column chunks so step-0 gates can start early
            thv = d_th.rearrange("k p j -> p k j")
            for mc in range(GM):
                cs = slice(mc * 128, (mc + 1) * 128)
                nc.sync.dma_start(sb_th[:, :, cs], thv[:, :, cs])
            x_first = xp.tile([128, FT, NLOC], fp16, tag="x", name="x_first")
            nc.scalar.dma_start(x_first, d_x[0].rearrange("f p n -> p f n"))
            for kg in range((KOWN + KT) // 2):
                ks = slice(kg * 2, (kg + 1) * 2)
                nc.sync.dma_start(
                    sb_ll2[:, ks, :], d_ll2[ks].rearrange("k p j -> p k j"))

            h_fm = None    # current H_i, feature-major [128, FT, NLOC] fp16
            c_fm = None    # current C_i, feature-major fp32
            hnm8_prev = None  # own H slice, node-major fp8 [128, KOWN, F]

            first_iter = True
            gacc = None
            for t in [tt for _r in range(repeat) for tt in range(T)]:
                if first_iter:
                    x_t = x_first
                    first_iter = False
                    gacc = gp.tile([128, GM, NLOC], fp32, tag="g", name="g0")
                    # ---- t=0: X-part inline (H is zero) ----
                    for m in range(GM):
                        pss = [
                            psg.tile([128, c1 - c0], fp32, tag="gps",
                                     name=f"gx0_{m}_{ci}")
                            for ci, (c0, c1) in enumerate(NCH)
                        ]
                        for i, kk in enumerate((0, 1)):
                            for ci, (c0, c1) in enumerate(NCH):
                                nc.tensor.matmul(
                                    pss[ci],
                                    sb_th[:, kk, m * 128:(m + 1) * 128],
                                    x_t[:, kk, c0:c1],
                                    start=(i == 0), stop=(i == 1))
                        for ci, (c0, c1) in enumerate(NCH):
                            if m % 2 == 0:
                                nc.scalar.activation(
                                    gacc[:, m, c0:c1], pss[ci],
                                    mybir.ActivationFunctionType.Copy)
                            else:
                                nc.vector.tensor_copy(gacc[:, m, c0:c1],
                                                      pss[ci])
                else:
                    # ---- gate matmul, H part (fp16; X-part was done
                    # during the previous step's LSTM window) ----
                    for m in range(GM):
                        pss = [
                            psg.tile([128, c1 - c0], fp32, tag="gps",
                                     name=f"gh{t}_{m}_{ci}")
                            for ci, (c0, c1) in enumerate(NCH)
                        ]
                        for i, kk in enumerate((2, 3)):
                            for ci, (c0, c1) in enumerate(NCH):
                                nc.tensor.matmul(
                                    pss[ci],
                                    sb_th[:, kk, m * 128:(m + 1) * 128],
                                    h_fm[:, kk - 2, c0:c1],
                                    start=(i == 0), stop=(i == 1))
                        for ci, (c0, c1) in enumerate(NCH):
                            nc.vector.tensor_add(
                                gacc[:, m, c0:c1], gacc[:, m, c0:c1],
                                pss[ci])

                if t > 0:
                    # ---- mega-prop: fp8 DoubleRow, own pairs first ----
                    u_fm = uvp.tile([128, FT, NLOC], fp8, tag="u", name=f"u{t}")
                    v_fm = uvp.tile([128, FT, NLOC], fp8, tag="v", name=f"v{t}")
                    for m in range(FT):
                        ms = slice(m * 128, (m + 1) * 128)
                        pps = [
                            psp.tile([128, p1 - p0], fp32, tag="pps",
                                     name=f"pps{t}_{m}_{ci}")
                            for ci, (p0, p1) in enumerate(PCH)
                        ]
                        # own pairs first: they only need the locally
                        # produced hnm8 (no AllGather), keeping the PE
                        # busy + HAM warm through the collective latency
                        for k in range(KPO):
                            ksl = slice(2 * k, 2 * k + 2)
                            for ci, (p0, p1) in enumerate(PCH):
                                nc.tensor.matmul(
                                    pps[ci], hnm8_prev[:, ksl, ms],
                                    sb_ll2[:, ksl, p0:p1],
                                    start=(k == 0), stop=False,
                                    perf_mode=DR)
                        for k in range(KPF):
                            ksl = slice(2 * k, 2 * k + 2)
                            lsl = slice(KOWN + 2 * k, KOWN + 2 * k + 2)
                            for ci, (p0, p1) in enumerate(PCH):
                                nc.tensor.matmul(
                                    pps[ci], sb_hfull[:, ksl, ms],
                                    sb_ll2[:, lsl, p0:p1],
                                    start=False, stop=(k == KPF - 1),
                                    perf_mode=DR)
                        # rescale copies on scalar: the vector engine's
                        # UV-adds + LSTM chain is the critical tail
                        ru, rv = sb_ruv[:, 0:1], sb_ruv[:, 1:2]
                        CP = mybir.ActivationFunctionType.Copy
                        nc.scalar.activation(u_fm[:, m, 0:512], pps[0], CP,
                                             scale=ru)
                        nc.scalar.activation(u_fm[:, m, 512:640],
                                             pps[1][:, 0:128], CP, scale=ru)
                        nc.scalar.activation(v_fm[:, m, 0:384],
                                             pps[1][:, 128:512], CP, scale=rv)
                        nc.scalar.activation(v_fm[:, m, 384:640], pps[2], CP,
                                             scale=rv)

                    # ---- gate matmul, U/V part (fp8 DoubleRow) ----
                    # even m-tiles first: they feed the ft=0 half of the LSTM,
                    # unblocking the first AllGather half earlier
                    for m in (0, 2, 4, 6, 1, 3, 5, 7):
                        ms = slice(m * 128, (m + 1) * 128)
                        pss = [
                            psg.tile([128, c1 - c0], fp32, tag="gps",
                                     name=f"guv{t}_{m}_{ci}")
                            for ci, (c0, c1) in enumerate(NCH)
                        ]
                        for ci, (c0, c1) in enumerate(NCH):
                            nc.tensor.matmul(
                                pss[ci], sb_th8[:, 0:2, ms],
                                u_fm[:, 0:2, c0:c1],
                                start=True, stop=False, perf_mode=DR)
                            nc.tensor.matmul(
                                pss[ci], sb_th8[:, 2:4, ms],
                                v_fm[:, 0:2, c0:c1],
                                start=False, stop=True, perf_mode=DR)
                        for ci, (c0, c1) in enumerate(NCH):
                            nc.vector.tensor_add(
                                gacc[:, m, c0:c1], gacc[:, m, c0:c1], pss[ci])

                # ---- LSTM cell (feature-major, elementwise), then transpose
                # the fresh H slice and kick the feature-half AllGathers ----
                last = (t == T - 1)
                h_new = hp.tile([128, FT, NLOC], fp32 if last else fp16,
                                tag="h32" if last else "h", name=f"h{t + 1}",
                                bufs=1 if last else None)
                c_new = hp.tile([128, FT, NLOC], fp32, tag="c", name=f"c{t + 1}")
                if not last:
                    hnm = hnmp.tile([128, 5, F], fp16, tag="hnm", name=f"hnm{t}")
                    hnm8 = hnmp.tile([128, KOWN, F], fp8, tag="hnm8",
                                     name=f"hnm8{t}")
                    nc.gpsimd.memset(hnm8[:, 5, :], 0)
                    agins, agouts = [], []
                    nag = FT if split_ag else 1
                    agw = 128 if split_ag else F
                    for ft in range(nag):
                        agins.append(dramp.tile(
                            [NLOC, agw], fp8, tag=f"agin{ft}",
                            name=f"agin{t}_{ft}"))
                        agouts.append(dramp.tile(
                            [N, agw], fp8, tag=f"agout{ft}",
                            addr_space="Shared", name=f"agout{t}_{ft}"))

                def emit_ag(ft):
                    fs = slice(ft * agw, (ft + 1) * agw)
                    # ft0 convert on gpsimd (idle, right before its AG);
                    # ft1 on vector so its AG issue isn't queued behind
                    # AG0's transfer blocking the gpsimd queue
                    eng = nc.gpsimd if ft == 0 else nc.vector
                    eng.tensor_scalar_mul(
                        hnm8[:, 0:5, fs], hnm[:, :, fs], SH)
                    nc.sync.dma_start(
                        agins[ft].rearrange("(k p) f -> p k f", p=128),
                        hnm8[:, 0:5, fs])
                    if not no_comm:
                        nc.gpsimd.collective_compute(
                            "AllGather",
                            mybir.AluOpType.bypass,
                            replica_groups=[list(range(NCORES))],
                            ins=[agins[ft].opt()],
                            outs=[agouts[ft].opt()],
                        )
                    agv = agouts[ft].rearrange("(k p) f -> p k f", p=128)
                    # first chunk small so the first foreign prop pair can
                    # start as early as possible after the AllGather lands
                    for k0, k1 in ((0, 2), (2, 6), (6, 14), (14, 24),
                                   (24, 32), (32, 40)):
                        nc.sync.dma_start(sb_hfull[:, k0:k1, fs],
                                          agv[:, k0:k1, :])
                # all 8 gate activations first (the ft1 set is then not
                # queued behind ft0's tanh(C) cross-engine wait)
                tis, tfs, tts, tos = [], [], [], []
                for ft in range(FT):
                    ti = tmpp.tile([128, NLOC], fp16, tag=f"t1{ft}",
                                   name=f"ti{t}_{ft}")
                    tf = tmpp.tile([128, NLOC], fp16, tag=f"t2{ft}",
                                   name=f"tf{t}_{ft}")
                    tt = tmpp.tile([128, NLOC], fp16, tag=f"t3{ft}",
                                   name=f"tt{t}_{ft}")
                    to = tmpp.tile([128, NLOC], fp16, tag=f"t4{ft}",
                                   name=f"to{t}_{ft}")
                    nc.scalar.activation(ti, gacc[:, 0 + ft, :], SIG,
                                         bias=sb_bias[:, 0 + ft:1 + ft],
                                         scale=1.0 / G)
                    nc.scalar.activation(tf, gacc[:, 2 + ft, :], SIG,
                                         bias=sb_bias[:, 2 + ft:3 + ft],
                                         scale=1.0 / G)
                    nc.scalar.activation(tt, gacc[:, 4 + ft, :], TANH,
                                         bias=sb_bias[:, 4 + ft:5 + ft],
                                         scale=1.0 / G)
                    nc.scalar.activation(to, gacc[:, 6 + ft, :], SIG,
                                         bias=sb_bias[:, 6 + ft:7 + ft],
                                         scale=1.0 / G)
                    tis.append(ti); tfs.append(tf)
                    tts.append(tt); tos.append(to)
                for ft in range(FT):
                    ti, tf, tt = tis[ft], tfs[ft], tts[ft]
                    if t == 0:
                        nc.vector.tensor_mul(c_new[:, ft, :], ti, tt)
                    else:
                        nc.vector.tensor_mul(ti, ti, tt)
                        nc.vector.tensor_mul(tf, tf, c_fm[:, ft, :])
                        nc.vector.tensor_add(c_new[:, ft, :], ti, tf)
                for ft in range(FT):
                    tc2 = tmpp.tile([128, NLOC], fp16, tag=f"t1{ft}",
                                    name=f"tc{t}_{ft}")
                    nc.scalar.activation(tc2, c_new[:, ft, :], TANH)
                    nc.vector.tensor_mul(h_new[:, ft, :], tos[ft], tc2)
                    if not last:
                        # node-major own slice (feature half ft)
                        fs = slice(ft * 128, (ft + 1) * 128)
                        nc.sync.dma_start_transpose(hnm[:, :, fs],
                                                    h_new[:, ft, :])
                        if split_ag:
                            emit_ag(ft)
                if not last and not split_ag:
                    emit_ag(0)

                # ---- X-part of next step's gates: pure PE filler that
                # runs during the LSTM/AllGather window (needs only the
                # prefetched x_{t+1}), keeping the PE busy and HAM warm
                if t < T - 1 or repeat > 1:
                    tn = t + 1 if t < T - 1 else 0
                    x_n = xp.tile([128, FT, NLOC], fp16, tag="x",
                                  name=f"x{t}_n")
                    nc.scalar.dma_start(x_n,
                                        d_x[tn].rearrange("f p n -> p f n"))
                    gacc_n = gp.tile([128, GM, NLOC], fp32, tag="g",
                                     name=f"g{t}_n")
                    for m in range(GM):
                        pss = [
                            psg.tile([128, c1 - c0], fp32, tag="gps",
                                     name=f"gx{t}n_{m}_{ci}")
                            for ci, (c0, c1) in enumerate(NCH)
                        ]
                        for i, kk in enumerate((0, 1)):
                            for ci, (c0, c1) in enumerate(NCH):
                                nc.tensor.matmul(
                                    pss[ci],
                                    sb_th[:, kk, m * 128:(m + 1) * 128],
                                    x_n[:, kk, c0:c1],
                                    start=(i == 0), stop=(i == 1))
                        for ci, (c0, c1) in enumerate(NCH):
                            if m % 2 == 0:
                                nc.scalar.activation(
                                    gacc_n[:, m, c0:c1], pss[ci],
                                    mybir.ActivationFunctionType.Copy)
                            else:
                                nc.vector.tensor_copy(gacc_n[:, m, c0:c1],
                                                      pss[ci])
                    x_t = x_n
                    gacc = gacc_n
                h_fm, c_fm = h_new, c_new
                if not last:
                    hnm8_prev = hnm8

            nc.sync.dma_start(d_h.rearrange("f p n -> p f n"), h_fm)
            nc.sync.dma_start(d_c.rearrange("f p n -> p f n"), c_fm)

    nc.compile()
    return nc


def _host_prep(X, edge_weight, Ws, bs, thetas, conv_bs, edge_index):
    """Build per-core device inputs from the raw problem inputs."""
    src = edge_index[0].astype(np.int64)
    dst = edge_index[1].astype(np.int64)
    ew = edge_weight.astype(np.float32)
    deg = np.bincount(src, weights=ew, minlength=N)
    dis = np.where(deg > 0, 1.0 / np.sqrt(np.where(deg > 0, deg, 1.0)), 0.0)
    dis = dis.astype(np.float32)
    w_hat = ((2.0 / LAMBDA_MAX) * (-dis[src] * ew * dis[dst])).astype(np.float32)
    diag = np.float32(2.0 / LAMBDA_MAX - 1.0)
    L = np.zeros((N, N), np.float32)
    np.add.at(L, (dst, src), w_hat)
    if diag != 0.0:
        L[np.arange(N), np.arange(N)] += diag
    L2 = L @ L

    SL = 224.0 / max(float(np.abs(L).max()), 1e-30)
    SL2 = 224.0 / max(float(np.abs(L2).max()), 1e-30)
    ruv_t = np.broadcast_to(
        np.array([[SU / (SH * SL)], [SV / (SH * SL2)]], np.float32),
        (2, 128)).copy()

    # Theta: rows [X; H; U; V] x cols [I|F|T|O].  X/H ride in fp16 scaled
    # by G; U/V ride in the fp8 tensor scaled by ATH (so the G-scaled
    # PSUM contributions match: (SU*U)x(ATH*th) = G*(U*th)).
    Th = np.zeros((4 * F, 4 * F), np.float32)
    Th8 = np.zeros((2 * F, 4 * F), np.float32)
    bias_full = np.zeros(4 * F, np.float32)
    for g in range(4):
        cs = slice(g * F, (g + 1) * F)
        Th[0 * F:1 * F, cs] = Ws[g] * G
        Th[1 * F:2 * F, cs] = (thetas[g, 0] - thetas[g, 2]) * G
        Th8[0 * F:1 * F, cs] = thetas[g, 1] * ATH
        Th8[1 * F:2 * F, cs] = 2.0 * thetas[g, 2] * ATH
        bias_full[cs] = bs[g] + conv_bs[g]
    th_t = np.ascontiguousarray(Th.reshape(GM, 128, 4 * F).astype(np.float16))
    th8_t = np.ascontiguousarray(
        np.clip(Th8, -240.0, 240.0).reshape(4, 128, 4 * F).astype(np_fp8))
    bias_t = np.ascontiguousarray(bias_full.reshape(GM, 128).astype(np.float32))

    in_maps = []
    for i in range(NCORES):
        rows = slice(i * NLOC, (i + 1) * NLOC)
        rhs = np.concatenate([L[rows].T * SL, L2[rows].T * SL2], axis=1)
        rhs = np.clip(rhs, -240.0, 240.0)
        # own-first layout: 5 own tiles + 1 zero pad, then the full
        # 40-tile global block with the own rows zeroed
        own = rhs[rows].reshape(5, 128, NOUT2)
        rest = rhs.copy()
        rest[rows] = 0.0
        ll2 = np.ascontiguousarray(np.concatenate(
            [own, np.zeros((1, 128, NOUT2), np.float32),
             rest.reshape(KT, 128, NOUT2)], axis=0).astype(np_fp8))
        # reference uses Xs = X.reshape(N, T, F) (torch-.view semantics: raw
        # memory reinterpretation), node n's time series is row n of that view
        xi = np.ascontiguousarray(
            X.reshape(N, T, F)[rows].transpose(1, 2, 0)
            .reshape(T, FT, 128, NLOC).astype(np.float16))
        in_maps.append(dict(ll2=ll2, th=th_t, th8=th8_t, xall=xi,
                            biasv=bias_t, ruv=ruv_t))
    return in_maps


def kernel(X, edge_weight, Ws, bs, thetas, conv_bs, edge_index):
    X = np.asarray(X, dtype=np.float32)
    edge_weight = np.asarray(edge_weight, dtype=np.float32)
    Ws = np.asarray(Ws, dtype=np.float32)
    bs = np.asarray(bs, dtype=np.float32)
    thetas = np.asarray(thetas, dtype=np.float32)
    conv_bs = np.asarray(conv_bs, dtype=np.float32)
    edge_index = np.asarray(edge_index)

    in_maps = _host_prep(X, edge_weight, Ws, bs, thetas, conv_bs, edge_index)
    if "nc" not in _CACHE:
        _CACHE["nc"] = _build_nc()
    nc = _CACHE["nc"]
    res = run_bass_kernel_spmd(nc, in_maps, core_ids=list(range(NCORES)))

    H = np.empty((N, F), np.float32)
    C = np.empty((N, F), np.float32)
    for i in range(NCORES):
        rows = slice(i * NLOC, (i + 1) * NLOC)
        H[rows] = res.results[i]["hout"].reshape(F, NLOC).T
        C[rows] = res.results[i]["cout"].reshape(F, NLOC).T
    return H, C



# revision 6
# speedup vs baseline: 1.0805x; 1.0805x over previous
"""GCLSTM (ChebConv-gated LSTM) Trainium2 kernel, 8-core SPMD.

Algorithm notes
---------------
reference computes, per timestep t (T=24) over N=5120 graph nodes:
    gate_g = X_t @ Ws[g] + cheb(H, thetas[g]) + biases      (4 gates)
    cheb(H, th) = H@th0 + (L@H)@th1 + (2L(LH) - H)@th2      (K=3 Chebyshev)
with L the scaled-normalized graph Laplacian (5120x5120, sparse, here
densified).  The Chebyshev basis (U = L@H, V = L^2@H) is shared by all 4
gates, so per step we need exactly ONE dense "mega-prop" [U|V] = [L;L^2]@H
plus the gate matmuls.  Folding:
    gate_g = X_t@Ws[g] + H@(th0-th2) + U@th1 + V@(2*th2) + b
so all gate work is a single [X;H;U;V] (1024) x Theta (1024x1024) matmul.

Sharding: nodes are split across 8 cores (640 each; edges connect
arbitrary nodes, so each core holds the full [L;L^2] column block for its
output rows, resident in SBUF).  The mega-prop contracts over ALL 5120
nodes, so the full H (node-major) is re-assembled every step with two
feature-half AllGathers (~13us each on the critical path); everything
else stays core-local.

Precision: the mega-prop (74% of all PE cycles) runs in fp8-e4m3 with
DoubleRow perf mode (2 fp8 weights per PE cell -> 2 MACs/cycle):
  - [L;L^2] is static: host-quantized to fp8 with global scales SL/SL2.
  - H for the prop is quantized on-device to SH*H fp8 right before the
    AllGather (which therefore also moves half the bytes).  The prop
    PSUM result (SH*SL*U fp32) is rescaled back to exact U at the
    PSUM->SBUF copy, so the gate matmuls see unscaled fp16 U/V.
The gate matmuls stay fp16: X quantization is numerically catastrophic
(X dominates the gate pre-activations; fp8 X alone costs 1.3e-1 rel
err vs 6.8e-3 for the whole prop-fp8 scheme).

Schedule (from NTFF profile analysis): the step's critical chain is
AllGather -> prop -> UV-gates -> LSTM -> transpose/AG-prep.  To keep
the PE fed during the AllGather (and the HAM clock-gate warm), the
contraction is ordered own-nodes-first: ll2 holds [5 own tiles + 1 zero
pad + 40 global tiles with own rows zeroed], and the 3 own pairs per
output half multiply against the locally produced node-major H (hnm8,
no AllGather needed).  DMA queues are split (arrival DMAs on sync,
transposes/agin/x-loads on the scalar HWDGE queue), the fp8 converts
run on gpsimd right before the collectives they feed, and the gate
PSUM->gacc copies/adds run on gpsimd to unload the vector engine,
whose LSTM chain is otherwise on the critical path.
"""
import sys

for _p in ("/opt/trn_rl_repo",):
    if _p not in sys.path:
        sys.path.insert(0, _p)

import numpy as np
import concourse.bass as bass
import concourse.mybir as mybir
import concourse.tile as tile
from concourse import bacc
from concourse.bass_utils import run_bass_kernel_spmd

fp32 = mybir.dt.float32
fp16 = mybir.dt.float16
fp8 = mybir.dt.float8e4
np_fp8 = mybir.dt.np(fp8)

NCORES = 8
B, T, NTOW, F = 512, 24, 10, 256
N = B * NTOW                  # 5120 nodes
NLOC = N // NCORES            # 640 nodes per core
KT = N // 128                 # 40 contraction tiles over nodes
KOWN = 6                      # 5 own tiles + 1 zero pad (even for pairing)
KPO = KOWN // 2               # 3 own DoubleRow pairs
KPF = KT // 2                 # 20 foreign DoubleRow pairs
FT = F // 128                 # 2 feature tiles
GM = (4 * F) // 128           # 8 gate-feature m-tiles
NOUT2 = 2 * NLOC              # 1280 = [U|V] output columns per core
LAMBDA_MAX = 2.0

NCH = [(0, 512), (512, 640)]             # node chunks for gate matmuls
PCH = [(0, 512), (512, 1024), (1024, 1280)]  # [U|V] column chunks

SH = 1.0         # H -> fp8 scale; 1.0 so the fp16->fp8 convert is a pure
                 # cast the SWDGE DMA can do in-flight (power-of-two scaling
                 # doesn't change e4m3 relative precision for |H| < 1)
ATH = 1024.0     # theta U/V blocks -> fp8 (|2*th2| <= ~0.22 -> <=222)
SU = 32.0        # U -> fp8 (|U| <~ 3)
SV = 32.0        # V -> fp8 (|V| <~ 6)
G = ATH * SU     # common gate scale: X/H theta blocks (fp16) carry G,
assert G == ATH * SV  # undone for free by the LSTM activation scale

SIG = mybir.ActivationFunctionType.Sigmoid
TANH = mybir.ActivationFunctionType.Tanh
DR = mybir.MatmulPerfMode.DoubleRow

_CACHE = {}


def _build_nc(repeat=1, no_comm=False, split_ag=True):
    nc = bacc.Bacc(None, target_bir_lowering=False, num_devices=NCORES)
    d_ll2 = nc.dram_tensor("ll2", [KOWN + KT, 128, NOUT2], fp8,
                           kind="ExternalInput")
    d_th = nc.dram_tensor("th", [GM, 128, 4 * F], fp16, kind="ExternalInput")
    d_th8 = nc.dram_tensor("th8", [4, 128, 4 * F], fp8, kind="ExternalInput")
    d_x = nc.dram_tensor("xall", [T, FT, 128, NLOC], fp16, kind="ExternalInput")
    d_bias = nc.dram_tensor("biasv", [GM, 128], fp32, kind="ExternalInput")
    d_ruv = nc.dram_tensor("ruv", [2, 128], fp32, kind="ExternalInput")
    d_h = nc.dram_tensor("hout", [FT, 128, NLOC], fp32, kind="ExternalOutput")
    d_c = nc.dram_tensor("cout", [FT, 128, NLOC], fp32, kind="ExternalOutput")

    with tile.TileContext(nc) as tc:
        with (
            tc.tile_pool(name="const", bufs=1) as constp,
            tc.tile_pool(name="xp", bufs=2) as xp,
            tc.tile_pool(name="gp", bufs=2) as gp,
            tc.tile_pool(name="uvp", bufs=1) as uvp,
            tc.tile_pool(name="hp", bufs=2) as hp,
            tc.tile_pool(name="hnmp", bufs=2) as hnmp,
            tc.tile_pool(name="tmpp", bufs=1) as tmpp,
            tc.tile_pool(name="psg", bufs=4, space="PSUM") as psg,
            tc.tile_pool(name="psp", bufs=4, space="PSUM") as psp,
            tc.tile_pool(name="dramio", bufs=2, space="DRAM") as dramp,
        ):
            # ---- resident tensors ----
            sb_ll2 = constp.tile([128, KOWN + KT, NOUT2], fp8, tag="ll2")
            sb_th = constp.tile([128, GM, 4 * F], fp16, tag="th")
            sb_th8 = constp.tile([128, 4, 4 * F], fp8, tag="th8")
            nc.sync.dma_start(sb_th8, d_th8.rearrange("k p j -> p k j"))
            sb_bias = constp.tile([128, GM], fp32, tag="bias")
            sb_ruv = constp.tile([128, 2], fp32, tag="ruv")
            sb_hfull = constp.tile([128, KT, F], fp8, tag="hfull")
            nc.sync.dma_start(sb_bias, d_bias.rearrange("m p -> p m"))
            nc.sync.dma_start(sb_ruv, d_ruv.rearrange("m p -> p m"))
            # theta in column chunks so step-0 gates can start early
            thv = d_th.rearrange("k p j -> p k j")
            for mc in range(GM):
                cs = slice(mc * 128, (mc + 1) * 128)
                nc.sync.dma_start(sb_th[:, :, cs], thv[:, :, cs])
            x_first = xp.tile([128, FT, NLOC], fp16, tag="x", name="x_first")
            nc.scalar.dma_start(x_first, d_x[0].rearrange("f p n -> p f n"))
            for kg in range((KOWN + KT) // 2):
                ks = slice(kg * 2, (kg + 1) * 2)
                nc.sync.dma_start(
                    sb_ll2[:, ks, :], d_ll2[ks].rearrange("k p j -> p k j"))

            h_fm = None    # current H_i, feature-major [128, FT, NLOC] fp16
            c_fm = None    # current C_i, feature-major fp32
            hnm8_prev = None  # own H slice, node-major fp8 [128, KOWN, F]

            first_iter = True
            gacc = None
            for t in [tt for _r in range(repeat) for tt in range(T)]:
                if first_iter:
                    x_t = x_first
                    first_iter = False
                    gacc = gp.tile([128, GM, NLOC], fp32, tag="g", name="g0")
                    # ---- t=0: X-part inline (H is zero) ----
                    for m in range(GM):
                        pss = [
                            psg.tile([128, c1 - c0], fp32, tag="gps",
                                     name=f"gx0_{m}_{ci}")
                            for ci, (c0, c1) in enumerate(NCH)
                        ]
                        for i, kk in enumerate((0, 1)):
                            for ci, (c0, c1) in enumerate(NCH):
                                nc.tensor.matmul(
                                    pss[ci],
                                    sb_th[:, kk, m * 128:(m + 1) * 128],
                                    x_t[:, kk, c0:c1],
                                    start=(i == 0), stop=(i == 1))
                        for ci, (c0, c1) in enumerate(NCH):
                            if m % 2 == 0:
                                nc.scalar.activation(
                                    gacc[:, m, c0:c1], pss[ci],
                                    mybir.ActivationFunctionType.Copy)
                            else:
                                nc.vector.tensor_copy(gacc[:, m, c0:c1],
                                                      pss[ci])
                else:
                    # ---- gate matmul, H part (fp16; X-part was done
                    # during the previous step's LSTM window) ----
                    for m in range(GM):
                        pss = [
                            psg.tile([128, c1 - c0], fp32, tag="gps",
                                     name=f"gh{t}_{m}_{ci}")
                            for ci, (c0, c1) in enumerate(NCH)
                        ]
                        for i, kk in enumerate((2, 3)):
                            for ci, (c0, c1) in enumerate(NCH):
                                nc.tensor.matmul(
                                    pss[ci],
                                    sb_th[:, kk, m * 128:(m + 1) * 128],
                                    h_fm[:, kk - 2, c0:c1],
                                    start=(i == 0), stop=(i == 1))
                        for ci, (c0, c1) in enumerate(NCH):
                            nc.vector.tensor_add(
                                gacc[:, m, c0:c1], gacc[:, m, c0:c1],
                                pss[ci])

                if t > 0:
                    # ---- mega-prop: fp8 DoubleRow, own pairs first ----
                    u_fm = uvp.tile([128, FT, NLOC], fp8, tag="u", name=f"u{t}")
                    v_fm = uvp.tile([128, FT, NLOC], fp8, tag="v", name=f"v{t}")
                    for m in range(FT):
                        ms = slice(m * 128, (m + 1) * 128)
                        pps = [
                            psp.tile([128, p1 - p0], fp32, tag="pps",
                                     name=f"pps{t}_{m}_{ci}")
                            for ci, (p0, p1) in enumerate(PCH)
                        ]
                        # own pairs first: they only need the locally
                        # produced hnm8 (no AllGather), keeping the PE
                        # busy + HAM warm through the collective latency
                        for k in range(KPO):
                            ksl = slice(2 * k, 2 * k + 2)
                            for ci, (p0, p1) in enumerate(PCH):
                                nc.tensor.matmul(
                                    pps[ci], hnm8_prev[:, ksl, ms],
                                    sb_ll2[:, ksl, p0:p1],
                                    start=(k == 0), stop=False,
                                    perf_mode=DR)
                        for k in range(KPF):
                            ksl = slice(2 * k, 2 * k + 2)
                            lsl = slice(KOWN + 2 * k, KOWN + 2 * k + 2)
                            for ci, (p0, p1) in enumerate(PCH):
                                nc.tensor.matmul(
                                    pps[ci], sb_hfull[:, ksl, ms],
                                    sb_ll2[:, lsl, p0:p1],
                                    start=False, stop=(k == KPF - 1),
                                    perf_mode=DR)
                        # rescale copies on scalar: the vector engine's
                        # UV-adds + LSTM chain is the critical tail
                        ru, rv = sb_ruv[:, 0:1], sb_ruv[:, 1:2]
                        CP = mybir.ActivationFunctionType.Copy
                        nc.scalar.activation(u_fm[:, m, 0:512], pps[0], CP,
                                             scale=ru)
                        nc.scalar.activation(u_fm[:, m, 512:640],
                                             pps[1][:, 0:128], CP, scale=ru)
                        nc.scalar.activation(v_fm[:, m, 0:384],
                                             pps[1][:, 128:512], CP, scale=rv)
                        nc.scalar.activation(v_fm[:, m, 384:640], pps[2], CP,
                                             scale=rv)

                    # ---- gate matmul, U/V part (fp8 DoubleRow) ----
                    # even m-tiles first: they feed the ft=0 half of the LSTM,
                    # unblocking the first AllGather half earlier
                    for m in (0, 2, 4, 6, 1, 3, 5, 7):
                        ms = slice(m * 128, (m + 1) * 128)
                        pss = [
                            psg.tile([128, c1 - c0], fp32, tag="gps",
                                     name=f"guv{t}_{m}_{ci}")
                            for ci, (c0, c1) in enumerate(NCH)
                        ]
                        for ci, (c0, c1) in enumerate(NCH):
                            nc.tensor.matmul(
                                pss[ci], sb_th8[:, 0:2, ms],
                                u_fm[:, 0:2, c0:c1],
                                start=True, stop=False, perf_mode=DR)
                            nc.tensor.matmul(
                                pss[ci], sb_th8[:, 2:4, ms],
                                v_fm[:, 0:2, c0:c1],
                                start=False, stop=True, perf_mode=DR)
                        for ci, (c0, c1) in enumerate(NCH):
                            nc.vector.tensor_add(
                                gacc[:, m, c0:c1], gacc[:, m, c0:c1], pss[ci])

                # ---- LSTM cell (feature-major, elementwise), then transpose
                # the fresh H slice and kick the feature-half AllGathers ----
                last = (t == T - 1)
                h_new = hp.tile([128, FT, NLOC], fp32 if last else fp16,
                                tag="h32" if last else "h", name=f"h{t + 1}",
                                bufs=1 if last else None)
                c_new = hp.tile([128, FT, NLOC], fp32, tag="c", name=f"c{t + 1}")
                if not last:
                    # per-ft transpose targets (contiguous: faster XBAR
                    # transpose + one big descriptor for the cast DMA)
                    hnms = [hnmp.tile([128, 5, 128], fp16, tag=f"hnm{ft}",
                                      name=f"hnm{t}_{ft}") for ft in range(FT)]
                    hnm8 = hnmp.tile([128, KOWN, F], fp8, tag="hnm8",
                                     name=f"hnm8{t}")
                    nc.gpsimd.memset(hnm8[:, 5, :], 0)
                    agins, agouts = [], []
                    nag = FT if split_ag else 1
                    agw = 128 if split_ag else F
                    for ft in range(nag):
                        agins.append(dramp.tile(
                            [NLOC, agw], fp8, tag=f"agin{ft}",
                            name=f"agin{t}_{ft}"))
                        agouts.append(dramp.tile(
                            [N, agw], fp8, tag=f"agout{ft}",
                            addr_space="Shared", name=f"agout{t}_{ft}"))

                def emit_ag(ft):
                    fs = slice(ft * agw, (ft + 1) * agw)
                    aginv = agins[ft].rearrange("(k p) f -> p k f", p=128)
                    # fp16->fp8 cast happens inside the SWDGE DMA straight
                    # into the collective input (the old tensor_scalar_mul
                    # convert cost 7-10us/step of critical path)
                    nc.gpsimd.dma_start(aginv, hnms[ft])
                    if not no_comm:
                        nc.gpsimd.collective_compute(
                            "AllGather",
                            mybir.AluOpType.bypass,
                            replica_groups=[list(range(NCORES))],
                            ins=[agins[ft].opt()],
                            outs=[agouts[ft].opt()],
                        )
                    # own-node fp8 slice for next step's own-first prop
                    # pairs: reload from agin (plain HWDGE, off the
                    # critical path; only needed next step)
                    nc.scalar.dma_start(hnm8[:, 0:5, fs], aginv)
                    agv = agouts[ft].rearrange("(k p) f -> p k f", p=128)
                    # first chunk small so the first foreign prop pair can
                    # start as early as possible after the AllGather lands
                    for k0, k1 in ((0, 2), (2, 6), (6, 14), (14, 24),
                                   (24, 32), (32, 40)):
                        nc.sync.dma_start(sb_hfull[:, k0:k1, fs],
                                          agv[:, k0:k1, :])
                # all 8 gate activations first (the ft1 set is then not
                # queued behind ft0's tanh(C) cross-engine wait)
                tis, tfs, tts, tos = [], [], [], []
                for ft in range(FT):
                    ti = tmpp.tile([128, NLOC], fp16, tag=f"t1{ft}",
                                   name=f"ti{t}_{ft}")
                    tf = tmpp.tile([128, NLOC], fp16, tag=f"t2{ft}",
                                   name=f"tf{t}_{ft}")
                    tt = tmpp.tile([128, NLOC], fp16, tag=f"t3{ft}",
                                   name=f"tt{t}_{ft}")
                    to = tmpp.tile([128, NLOC], fp16, tag=f"t4{ft}",
                                   name=f"to{t}_{ft}")
                    nc.scalar.activation(ti, gacc[:, 0 + ft, :], SIG,
                                         bias=sb_bias[:, 0 + ft:1 + ft],
                                         scale=1.0 / G)
                    nc.scalar.activation(tf, gacc[:, 2 + ft, :], SIG,
                                         bias=sb_bias[:, 2 + ft:3 + ft],
                                         scale=1.0 / G)
                    nc.scalar.activation(tt, gacc[:, 4 + ft, :], TANH,
                                         bias=sb_bias[:, 4 + ft:5 + ft],
                                         scale=1.0 / G)
                    nc.scalar.activation(to, gacc[:, 6 + ft, :], SIG,
                                         bias=sb_bias[:, 6 + ft:7 + ft],
                                         scale=1.0 / G)
                    tis.append(ti); tfs.append(tf)
                    tts.append(tt); tos.append(to)
                for ft in range(FT):
                    ti, tf, tt = tis[ft], tfs[ft], tts[ft]
                    if t == 0:
                        nc.vector.tensor_mul(c_new[:, ft, :], ti, tt)
                    else:
                        nc.vector.tensor_mul(ti, ti, tt)
                        nc.vector.tensor_mul(tf, tf, c_fm[:, ft, :])
                        nc.vector.tensor_add(c_new[:, ft, :], ti, tf)
                for ft in range(FT):
                    tc2 = tmpp.tile([128, NLOC], fp16, tag=f"t1{ft}",
                                    name=f"tc{t}_{ft}")
                    nc.scalar.activation(tc2, c_new[:, ft, :], TANH)
                    nc.vector.tensor_mul(h_new[:, ft, :], tos[ft], tc2)
                    if not last:
                        # node-major own slice (feature half ft)
                        nc.sync.dma_start_transpose(hnms[ft],
                                                    h_new[:, ft, :])
                        if split_ag:
                            emit_ag(ft)
                if not last and not split_ag:
                    emit_ag(0)

                # ---- X-part of next step's gates: pure PE filler that
                # runs during the LSTM/AllGather window (needs only the
                # prefetched x_{t+1}), keeping the PE busy and HAM warm
                if t < T - 1 or repeat > 1:
                    tn = t + 1 if t < T - 1 else 0
                    x_n = xp.tile([128, FT, NLOC], fp16, tag="x",
                                  name=f"x{t}_n")
                    nc.scalar.dma_start(x_n,
                                        d_x[tn].rearrange("f p n -> p f n"))
                    gacc_n = gp.tile([128, GM, NLOC], fp32, tag="g",
                                     name=f"g{t}_n")
                    for m in range(GM):
                        pss = [
                            psg.tile([128, c1 - c0], fp32, tag="gps",
                                     name=f"gx{t}n_{m}_{ci}")
                            for ci, (c0, c1) in enumerate(NCH)
                        ]
                        for i, kk in enumerate((0, 1)):
                            for ci, (c0, c1) in enumerate(NCH):
                                nc.tensor.matmul(
                                    pss[ci],
                                    sb_th[:, kk, m * 128:(m + 1) * 128],
                                    x_n[:, kk, c0:c1],
                                    start=(i == 0), stop=(i == 1))
                        for ci, (c0, c1) in enumerate(NCH):
                            if m % 2 == 0:
                                nc.scalar.activation(
                                    gacc_n[:, m, c0:c1], pss[ci],
                                    mybir.ActivationFunctionType.Copy)
                            else:
                                nc.vector.tensor_copy(gacc_n[:, m, c0:c1],
                                                      pss[ci])
                    x_t = x_n
                    gacc = gacc_n
                h_fm, c_fm = h_new, c_new
                if not last:
                    hnm8_prev = hnm8

            nc.sync.dma_start(d_h.rearrange("f p n -> p f n"), h_fm)
            nc.sync.dma_start(d_c.rearrange("f p n -> p f n"), c_fm)

    nc.compile()
    return nc


def _host_prep(X, edge_weight, Ws, bs, thetas, conv_bs, edge_index):
    """Build per-core device inputs from the raw problem inputs."""
    src = edge_index[0].astype(np.int64)
    dst = edge_index[1].astype(np.int64)
    ew = edge_weight.astype(np.float32)
    deg = np.bincount(src, weights=ew, minlength=N)
    dis = np.where(deg > 0, 1.0 / np.sqrt(np.where(deg > 0, deg, 1.0)), 0.0)
    dis = dis.astype(np.float32)
    w_hat = ((2.0 / LAMBDA_MAX) * (-dis[src] * ew * dis[dst])).astype(np.float32)
    diag = np.float32(2.0 / LAMBDA_MAX - 1.0)
    L = np.zeros((N, N), np.float32)
    np.add.at(L, (dst, src), w_hat)
    if diag != 0.0:
        L[np.arange(N), np.arange(N)] += diag
    L2 = L @ L

    SL = 224.0 / max(float(np.abs(L).max()), 1e-30)
    SL2 = 224.0 / max(float(np.abs(L2).max()), 1e-30)
    ruv_t = np.broadcast_to(
        np.array([[SU / (SH * SL)], [SV / (SH * SL2)]], np.float32),
        (2, 128)).copy()

    # Theta: rows [X; H; U; V] x cols [I|F|T|O].  X/H ride in fp16 scaled
    # by G; U/V ride in the fp8 tensor scaled by ATH (so the G-scaled
    # PSUM contributions match: (SU*U)x(ATH*th) = G*(U*th)).
    Th = np.zeros((4 * F, 4 * F), np.float32)
    Th8 = np.zeros((2 * F, 4 * F), np.float32)
    bias_full = np.zeros(4 * F, np.float32)
    for g in range(4):
        cs = slice(g * F, (g + 1) * F)
        Th[0 * F:1 * F, cs] = Ws[g] * G
        Th[1 * F:2 * F, cs] = (thetas[g, 0] - thetas[g, 2]) * G
        Th8[0 * F:1 * F, cs] = thetas[g, 1] * ATH
        Th8[1 * F:2 * F, cs] = 2.0 * thetas[g, 2] * ATH
        bias_full[cs] = bs[g] + conv_bs[g]
    th_t = np.ascontiguousarray(Th.reshape(GM, 128, 4 * F).astype(np.float16))
    th8_t = np.ascontiguousarray(
        np.clip(Th8, -240.0, 240.0).reshape(4, 128, 4 * F).astype(np_fp8))
    bias_t = np.ascontiguousarray(bias_full.reshape(GM, 128).astype(np.float32))

    in_maps = []
    for i in range(NCORES):
        rows = slice(i * NLOC, (i + 1) * NLOC)
        rhs = np.concatenate([L[rows].T * SL, L2[rows].T * SL2], axis=1)
        rhs = np.clip(rhs, -240.0, 240.0)
        # own-first layout: 5 own tiles + 1 zero pad, then the full
        # 40-tile global block with the own rows zeroed
        own = rhs[rows].reshape(5, 128, NOUT2)
        rest = rhs.copy()
        rest[rows] = 0.0
        ll2 = np.ascontiguousarray(np.concatenate(
            [own, np.zeros((1, 128, NOUT2), np.float32),
             rest.reshape(KT, 128, NOUT2)], axis=0).astype(np_fp8))
        # reference uses Xs = X.reshape(N, T, F) (torch-.view semantics: raw
        # memory reinterpretation), node n's time series is row n of that view
        xi = np.ascontiguousarray(
            X.reshape(N, T, F)[rows].transpose(1, 2, 0)
            .reshape(T, FT, 128, NLOC).astype(np.float16))
        in_maps.append(dict(ll2=ll2, th=th_t, th8=th8_t, xall=xi,
                            biasv=bias_t, ruv=ruv_t))
    return in_maps


def kernel(X, edge_weight, Ws, bs, thetas, conv_bs, edge_index):
    X = np.asarray(X, dtype=np.float32)
    edge_weight = np.asarray(edge_weight, dtype=np.float32)
    Ws = np.asarray(Ws, dtype=np.float32)
    bs = np.asarray(bs, dtype=np.float32)
    thetas = np.asarray(thetas, dtype=np.float32)
    conv_bs = np.asarray(conv_bs, dtype=np.float32)
    edge_index = np.asarray(edge_index)

    in_maps = _host_prep(X, edge_weight, Ws, bs, thetas, conv_bs, edge_index)
    if "nc" not in _CACHE:
        _CACHE["nc"] = _build_nc()
    nc = _CACHE["nc"]
    res = run_bass_kernel_spmd(nc, in_maps, core_ids=list(range(NCORES)))

    H = np.empty((N, F), np.float32)
    C = np.empty((N, F), np.float32)
    for i in range(NCORES):
        rows = slice(i * NLOC, (i + 1) * NLOC)
        H[rows] = res.results[i]["hout"].reshape(F, NLOC).T
        C[rows] = res.results[i]["cout"].reshape(F, NLOC).T
    return H, C



# revision 11
# speedup vs baseline: 1.0844x; 1.0036x over previous
"""GCLSTM (ChebConv-gated LSTM) Trainium2 kernel, 8-core SPMD.

Algorithm notes
---------------
reference computes, per timestep t (T=24) over N=5120 graph nodes:
    gate_g = X_t @ Ws[g] + cheb(H, thetas[g]) + biases      (4 gates)
    cheb(H, th) = H@th0 + (L@H)@th1 + (2L(LH) - H)@th2      (K=3 Chebyshev)
with L the scaled-normalized graph Laplacian (5120x5120, sparse, here
densified).  The Chebyshev basis (U = L@H, V = L^2@H) is shared by all 4
gates, so per step we need exactly ONE dense "mega-prop" [U|V] = [L;L^2]@H
plus the gate matmuls.  Folding:
    gate_g = X_t@Ws[g] + H@(th0-th2) + U@th1 + V@(2*th2) + b
so all gate work is a single [X;H;U;V] (1024) x Theta (1024x1024) matmul.

Sharding: nodes are split across 8 cores (640 each; edges connect
arbitrary nodes, so each core holds the full [L;L^2] column block for its
output rows, resident in SBUF).  The mega-prop contracts over ALL 5120
nodes, so the full H (node-major) is re-assembled every step with two
feature-half AllGathers (~13us each on the critical path); everything
else stays core-local.

Precision: the mega-prop (74% of all PE cycles) runs in fp8-e4m3 with
DoubleRow perf mode (2 fp8 weights per PE cell -> 2 MACs/cycle):
  - [L;L^2] is static: host-quantized to fp8 with global scales SL/SL2.
  - H for the prop is quantized on-device to SH*H fp8 right before the
    AllGather (which therefore also moves half the bytes).  The prop
    PSUM result (SH*SL*U fp32) is rescaled back to exact U at the
    PSUM->SBUF copy, so the gate matmuls see unscaled fp16 U/V.
The gate matmuls stay fp16: X quantization is numerically catastrophic
(X dominates the gate pre-activations; fp8 X alone costs 1.3e-1 rel
err vs 6.8e-3 for the whole prop-fp8 scheme).

Schedule (from NTFF profile analysis): the step's critical chain is
AllGather -> prop -> UV-gates -> LSTM -> transpose/AG-prep.  To keep
the PE fed during the AllGather (and the HAM clock-gate warm), the
contraction is ordered own-nodes-first: ll2 holds [5 own tiles + 1 zero
pad + 40 global tiles with own rows zeroed], and the 3 own pairs per
output half multiply against the locally produced node-major H (hnm8,
no AllGather needed).  DMA queues are split (arrival DMAs on sync,
transposes/agin/x-loads on the scalar HWDGE queue), the fp8 converts
run on gpsimd right before the collectives they feed, and the gate
PSUM->gacc copies/adds run on gpsimd to unload the vector engine,
whose LSTM chain is otherwise on the critical path.
"""
import sys

for _p in ("/opt/trn_rl_repo",):
    if _p not in sys.path:
        sys.path.insert(0, _p)

import numpy as np
import concourse.bass as bass
import concourse.mybir as mybir
import concourse.tile as tile
from concourse import bacc
from concourse.bass_utils import run_bass_kernel_spmd

fp32 = mybir.dt.float32
fp16 = mybir.dt.float16
fp8 = mybir.dt.float8e4
np_fp8 = mybir.dt.np(fp8)

NCORES = 8
B, T, NTOW, F = 512, 24, 10, 256
N = B * NTOW                  # 5120 nodes
NLOC = N // NCORES            # 640 nodes per core
KT = N // 128                 # 40 contraction tiles over nodes
KOWN = 6                      # 5 own tiles + 1 zero pad (even for pairing)
KPO = KOWN // 2               # 3 own DoubleRow pairs
KPF = KT // 2                 # 20 foreign DoubleRow pairs
FT = F // 128                 # 2 feature tiles
GM = (4 * F) // 128           # 8 gate-feature m-tiles
NOUT2 = 2 * NLOC              # 1280 = [U|V] output columns per core
LAMBDA_MAX = 2.0

NCH = [(0, 512), (512, 640)]             # node chunks for gate matmuls
PCH = [(0, 512), (512, 1024), (1024, 1280)]  # [U|V] column chunks

SH = 1.0         # H -> fp8 scale; 1.0 so the fp16->fp8 convert is a pure
                 # cast the SWDGE DMA can do in-flight (power-of-two scaling
                 # doesn't change e4m3 relative precision for |H| < 1)
ATH = 1024.0     # theta U/V blocks -> fp8 (|2*th2| <= ~0.22 -> <=222)
SU = 32.0        # U -> fp8 (|U| <~ 3)
SV = 32.0        # V -> fp8 (|V| <~ 6)
G = ATH * SU     # common gate scale: X/H theta blocks (fp16) carry G,
assert G == ATH * SV  # undone for free by the LSTM activation scale

SIG = mybir.ActivationFunctionType.Sigmoid
TANH = mybir.ActivationFunctionType.Tanh
DR = mybir.MatmulPerfMode.DoubleRow

_CACHE = {}


def _build_nc(repeat=1, no_comm=False, split_ag=True):
    nc = bacc.Bacc(None, target_bir_lowering=False, num_devices=NCORES)
    d_ll2 = nc.dram_tensor("ll2", [KOWN + KT, 128, NOUT2], fp8,
                           kind="ExternalInput")
    d_th = nc.dram_tensor("th", [GM, 128, 4 * F], fp16, kind="ExternalInput")
    d_th8 = nc.dram_tensor("th8", [4, 128, 4 * F], fp8, kind="ExternalInput")
    d_x = nc.dram_tensor("xall", [T, FT, 128, NLOC], fp16, kind="ExternalInput")
    d_bias = nc.dram_tensor("biasv", [GM, 128], fp32, kind="ExternalInput")
    d_ruv = nc.dram_tensor("ruv", [2, 128], fp32, kind="ExternalInput")
    d_h = nc.dram_tensor("hout", [FT, 128, NLOC], fp32, kind="ExternalOutput")
    d_c = nc.dram_tensor("cout", [FT, 128, NLOC], fp32, kind="ExternalOutput")

    with tile.TileContext(nc) as tc:
        with (
            tc.tile_pool(name="const", bufs=1) as constp,
            tc.tile_pool(name="xp", bufs=2) as xp,
            tc.tile_pool(name="gp", bufs=2) as gp,
            tc.tile_pool(name="uvp", bufs=1) as uvp,
            tc.tile_pool(name="hp", bufs=2) as hp,
            tc.tile_pool(name="hnmp", bufs=2) as hnmp,
            tc.tile_pool(name="tmpp", bufs=1) as tmpp,
            tc.tile_pool(name="psg", bufs=4, space="PSUM") as psg,
            tc.tile_pool(name="psp", bufs=4, space="PSUM") as psp,
            tc.tile_pool(name="dramio", bufs=2, space="DRAM") as dramp,
        ):
            # ---- resident tensors ----
            sb_ll2 = constp.tile([128, KOWN + KT, NOUT2], fp8, tag="ll2")
            sb_th = constp.tile([128, GM, 4 * F], fp16, tag="th")
            sb_th8 = constp.tile([128, 4, 4 * F], fp8, tag="th8")
            nc.sync.dma_start(sb_th8, d_th8.rearrange("k p j -> p k j"))
            sb_bias = constp.tile([128, GM], fp32, tag="bias")
            sb_ruv = constp.tile([128, 2], fp32, tag="ruv")
            # ft-major so each AllGather-chunk DMA writes contiguous
            # per-partition runs (k*128B descriptors instead of 128B)
            sb_hfull = constp.tile([128, FT, KT, 128], fp8, tag="hfull")
            nc.sync.dma_start(sb_bias, d_bias.rearrange("m p -> p m"))
            nc.sync.dma_start(sb_ruv, d_ruv.rearrange("m p -> p m"))
            # theta in column chunks so step-0 gates can start early
            thv = d_th.rearrange("k p j -> p k j")
            for mc in range(GM):
                cs = slice(mc * 128, (mc + 1) * 128)
                nc.sync.dma_start(sb_th[:, :, cs], thv[:, :, cs])
            x_first = xp.tile([128, FT, NLOC], fp16, tag="x", name="x_first")
            nc.scalar.dma_start(x_first, d_x[0].rearrange("f p n -> p f n"))
            for kg in range((KOWN + KT) // 2):
                ks = slice(kg * 2, (kg + 1) * 2)
                nc.sync.dma_start(
                    sb_ll2[:, ks, :], d_ll2[ks].rearrange("k p j -> p k j"))

            h_fm = None    # current H_i, feature-major [128, FT, NLOC] fp16
            c_fm = None    # current C_i, feature-major fp32
            hnm8_prev = None  # own H slice, node-major fp8 [128, KOWN, F]

            first_iter = True
            gacc = None
            for t in [tt for _r in range(repeat) for tt in range(T)]:
                if first_iter:
                    x_t = x_first
                    first_iter = False
                    gacc = gp.tile([128, GM, NLOC], fp32, tag="g", name="g0")
                    # ---- t=0: X-part inline (H is zero) ----
                    for m in range(GM):
                        pss = [
                            psg.tile([128, c1 - c0], fp32, tag="gps",
                                     name=f"gx0_{m}_{ci}")
                            for ci, (c0, c1) in enumerate(NCH)
                        ]
                        for i, kk in enumerate((0, 1)):
                            for ci, (c0, c1) in enumerate(NCH):
                                nc.tensor.matmul(
                                    pss[ci],
                                    sb_th[:, kk, m * 128:(m + 1) * 128],
                                    x_t[:, kk, c0:c1],
                                    start=(i == 0), stop=(i == 1))
                        for ci, (c0, c1) in enumerate(NCH):
                            if m % 2 == 0:
                                nc.scalar.activation(
                                    gacc[:, m, c0:c1], pss[ci],
                                    mybir.ActivationFunctionType.Copy)
                            else:
                                nc.vector.tensor_copy(gacc[:, m, c0:c1],
                                                      pss[ci])
                else:
                    # ---- gate matmul, H part (fp16; X-part was done
                    # during the previous step's LSTM window) ----
                    for m in range(GM):
                        pss = [
                            psg.tile([128, c1 - c0], fp32, tag="gps",
                                     name=f"gh{t}_{m}_{ci}")
                            for ci, (c0, c1) in enumerate(NCH)
                        ]
                        for i, kk in enumerate((2, 3)):
                            for ci, (c0, c1) in enumerate(NCH):
                                nc.tensor.matmul(
                                    pss[ci],
                                    sb_th[:, kk, m * 128:(m + 1) * 128],
                                    h_fm[:, kk - 2, c0:c1],
                                    start=(i == 0), stop=(i == 1))
                        for ci, (c0, c1) in enumerate(NCH):
                            nc.vector.tensor_add(
                                gacc[:, m, c0:c1], gacc[:, m, c0:c1],
                                pss[ci])

                if t > 0:
                    # ---- mega-prop: fp8 DoubleRow, own pairs first ----
                    u_fm = uvp.tile([128, FT, NLOC], fp8, tag="u", name=f"u{t}")
                    v_fm = uvp.tile([128, FT, NLOC], fp8, tag="v", name=f"v{t}")
                    for m in range(FT):
                        ms = slice(m * 128, (m + 1) * 128)
                        pps = [
                            psp.tile([128, p1 - p0], fp32, tag="pps",
                                     name=f"pps{t}_{m}_{ci}")
                            for ci, (p0, p1) in enumerate(PCH)
                        ]
                        # own pairs first: they only need the locally
                        # produced hnm8 (no AllGather), keeping the PE
                        # busy + HAM warm through the collective latency
                        for k in range(KPO):
                            ksl = slice(2 * k, 2 * k + 2)
                            for ci, (p0, p1) in enumerate(PCH):
                                nc.tensor.matmul(
                                    pps[ci], hnm8_prev[:, ksl, ms],
                                    sb_ll2[:, ksl, p0:p1],
                                    start=(k == 0), stop=False,
                                    perf_mode=DR)
                        for k in range(KPF):
                            ksl = slice(2 * k, 2 * k + 2)
                            lsl = slice(KOWN + 2 * k, KOWN + 2 * k + 2)
                            for ci, (p0, p1) in enumerate(PCH):
                                nc.tensor.matmul(
                                    pps[ci], sb_hfull[:, m, ksl, :],
                                    sb_ll2[:, lsl, p0:p1],
                                    start=False, stop=(k == KPF - 1),
                                    perf_mode=DR)
                        # rescale copies on scalar: the vector engine's
                        # UV-adds + LSTM chain is the critical tail
                        ru, rv = sb_ruv[:, 0:1], sb_ruv[:, 1:2]
                        CP = mybir.ActivationFunctionType.Copy
                        nc.scalar.activation(u_fm[:, m, 0:512], pps[0], CP,
                                             scale=ru)
                        nc.scalar.activation(u_fm[:, m, 512:640],
                                             pps[1][:, 0:128], CP, scale=ru)
                        nc.scalar.activation(v_fm[:, m, 0:384],
                                             pps[1][:, 128:512], CP, scale=rv)
                        nc.scalar.activation(v_fm[:, m, 384:640], pps[2], CP,
                                             scale=rv)

                    # ---- gate matmul, U/V part (fp8 DoubleRow) ----
                    # even m-tiles first: they feed the ft=0 half of the LSTM,
                    # unblocking the first AllGather half earlier
                    for m in (0, 2, 4, 6, 1, 3, 5, 7):
                        ms = slice(m * 128, (m + 1) * 128)
                        pss = [
                            psg.tile([128, c1 - c0], fp32, tag="gps",
                                     name=f"guv{t}_{m}_{ci}")
                            for ci, (c0, c1) in enumerate(NCH)
                        ]
                        # U over both column chunks, then V: consecutive
                        # matmuls share the stationary operand so its
                        # LDWEIGHTS stays hidden (U,V,U,V exposed ~160ns
                        # of DR weight load per matmul)
                        for ci, (c0, c1) in enumerate(NCH):
                            nc.tensor.matmul(
                                pss[ci], sb_th8[:, 0:2, ms],
                                u_fm[:, 0:2, c0:c1],
                                start=True, stop=False, perf_mode=DR)
                        for ci, (c0, c1) in enumerate(NCH):
                            nc.tensor.matmul(
                                pss[ci], sb_th8[:, 2:4, ms],
                                v_fm[:, 0:2, c0:c1],
                                start=False, stop=True, perf_mode=DR)
                        for ci, (c0, c1) in enumerate(NCH):
                            nc.vector.tensor_add(
                                gacc[:, m, c0:c1], gacc[:, m, c0:c1], pss[ci])

                # ---- LSTM cell (feature-major, elementwise), then transpose
                # the fresh H slice and kick the feature-half AllGathers ----
                last = (t == T - 1)
                h_new = hp.tile([128, FT, NLOC], fp32 if last else fp16,
                                tag="h32" if last else "h", name=f"h{t + 1}",
                                bufs=1 if last else None)
                c_new = hp.tile([128, FT, NLOC], fp32, tag="c", name=f"c{t + 1}")
                if not last:
                    # per-ft transpose targets (contiguous: faster XBAR
                    # transpose + one big descriptor for the cast DMA)
                    hnms = [hnmp.tile([128, 5, 128], fp16, tag=f"hnm{ft}",
                                      name=f"hnm{t}_{ft}") for ft in range(FT)]
                    hnm8 = hnmp.tile([128, KOWN, F], fp8, tag="hnm8",
                                     name=f"hnm8{t}")
                    nc.gpsimd.memset(hnm8[:, 5, :], 0)
                    agins, agouts = [], []
                    nag = FT if split_ag else 1
                    agw = 128 if split_ag else F
                    for ft in range(nag):
                        agins.append(dramp.tile(
                            [NLOC, agw], fp8, tag=f"agin{ft}",
                            name=f"agin{t}_{ft}"))
                        agouts.append(dramp.tile(
                            [N, agw], fp8, tag=f"agout{ft}",
                            addr_space="Shared", name=f"agout{t}_{ft}"))

                def emit_ag(ft):
                    aginv = agins[ft].rearrange("(k p) f -> p k f", p=128)
                    # fp16->fp8 cast happens inside the SWDGE DMA straight
                    # into the collective input (the old tensor_scalar_mul
                    # convert cost 7-10us/step of critical path)
                    nc.gpsimd.dma_start(aginv, hnms[ft])
                    if not no_comm:
                        nc.gpsimd.collective_compute(
                            "AllGather",
                            mybir.AluOpType.bypass,
                            replica_groups=[list(range(NCORES))],
                            ins=[agins[ft].opt()],
                            outs=[agouts[ft].opt()],
                        )
                    agv = agouts[ft].rearrange("(k p) f -> p k f", p=128)
                    # first chunk small so the first foreign prop pair can
                    # start as early as possible after the AllGather lands
                    for k0, k1 in ((0, 2), (2, 6), (6, 16), (16, 28),
                                   (28, 40)):
                        nc.sync.dma_start(sb_hfull[:, ft, k0:k1, :],
                                          agv[:, k0:k1, :])
                # all 8 gate activations first (the ft1 set is then not
                # queued behind ft0's tanh(C) cross-engine wait)
                tis, tfs, tts, tos = [], [], [], []
                for ft in range(FT):
                    ti = tmpp.tile([128, NLOC], fp16, tag=f"t1{ft}",
                                   name=f"ti{t}_{ft}")
                    tf = tmpp.tile([128, NLOC], fp16, tag=f"t2{ft}",
                                   name=f"tf{t}_{ft}")
                    tt = tmpp.tile([128, NLOC], fp16, tag=f"t3{ft}",
                                   name=f"tt{t}_{ft}")
                    to = tmpp.tile([128, NLOC], fp16, tag=f"t4{ft}",
                                   name=f"to{t}_{ft}")
                    nc.scalar.activation(ti, gacc[:, 0 + ft, :], SIG,
                                         bias=sb_bias[:, 0 + ft:1 + ft],
                                         scale=1.0 / G)
                    nc.scalar.activation(tf, gacc[:, 2 + ft, :], SIG,
                                         bias=sb_bias[:, 2 + ft:3 + ft],
                                         scale=1.0 / G)
                    nc.scalar.activation(tt, gacc[:, 4 + ft, :], TANH,
                                         bias=sb_bias[:, 4 + ft:5 + ft],
                                         scale=1.0 / G)
                    nc.scalar.activation(to, gacc[:, 6 + ft, :], SIG,
                                         bias=sb_bias[:, 6 + ft:7 + ft],
                                         scale=1.0 / G)
                    tis.append(ti); tfs.append(tf)
                    tts.append(tt); tos.append(to)
                for ft in range(FT):
                    ti, tf, tt = tis[ft], tfs[ft], tts[ft]
                    if t == 0:
                        nc.vector.tensor_mul(c_new[:, ft, :], ti, tt)
                    else:
                        nc.vector.tensor_mul(ti, ti, tt)
                        nc.vector.tensor_mul(tf, tf, c_fm[:, ft, :])
                        nc.vector.tensor_add(c_new[:, ft, :], ti, tf)
                for ft in range(FT):
                    tc2 = tmpp.tile([128, NLOC], fp16, tag=f"t1{ft}",
                                    name=f"tc{t}_{ft}")
                    nc.scalar.activation(tc2, c_new[:, ft, :], TANH)
                    nc.vector.tensor_mul(h_new[:, ft, :], tos[ft], tc2)
                    if not last:
                        # node-major own slice (feature half ft); scalar
                        # HWDGE queue is shallow (the sync queue's chunk
                        # DMAs added ~3.5us of issue latency here)
                        nc.scalar.dma_start_transpose(hnms[ft],
                                                      h_new[:, ft, :])
                        if split_ag:
                            emit_ag(ft)
                if not last and not split_ag:
                    emit_ag(0)
                if not last:
                    # own-node fp8 slices for next step's own-first prop
                    # pairs: SBUF->SBUF cast DMAs, emitted after both AG
                    # triggers so they never delay the collectives
                    for ft in range(FT):
                        fs = slice(ft * 128, (ft + 1) * 128)
                        nc.gpsimd.dma_start(hnm8[:, 0:5, fs], hnms[ft])

                # ---- X-part of next step's gates: pure PE filler that
                # runs during the LSTM/AllGather window (needs only the
                # prefetched x_{t+1}), keeping the PE busy and HAM warm
                if t < T - 1 or repeat > 1:
                    tn = t + 1 if t < T - 1 else 0
                    x_n = xp.tile([128, FT, NLOC], fp16, tag="x",
                                  name=f"x{t}_n")
                    nc.scalar.dma_start(x_n,
                                        d_x[tn].rearrange("f p n -> p f n"))
                    gacc_n = gp.tile([128, GM, NLOC], fp32, tag="g",
                                     name=f"g{t}_n")
                    for m in range(GM):
                        pss = [
                            psg.tile([128, c1 - c0], fp32, tag="gps",
                                     name=f"gx{t}n_{m}_{ci}")
                            for ci, (c0, c1) in enumerate(NCH)
                        ]
                        for i, kk in enumerate((0, 1)):
                            for ci, (c0, c1) in enumerate(NCH):
                                nc.tensor.matmul(
                                    pss[ci],
                                    sb_th[:, kk, m * 128:(m + 1) * 128],
                                    x_n[:, kk, c0:c1],
                                    start=(i == 0), stop=(i == 1))
                        for ci, (c0, c1) in enumerate(NCH):
                            if m % 2 == 0:
                                nc.scalar.activation(
                                    gacc_n[:, m, c0:c1], pss[ci],
                                    mybir.ActivationFunctionType.Copy)
                            else:
                                nc.vector.tensor_copy(gacc_n[:, m, c0:c1],
                                                      pss[ci])
                    x_t = x_n
                    gacc = gacc_n
                h_fm, c_fm = h_new, c_new
                if not last:
                    hnm8_prev = hnm8

            nc.sync.dma_start(d_h.rearrange("f p n -> p f n"), h_fm)
            nc.sync.dma_start(d_c.rearrange("f p n -> p f n"), c_fm)

    nc.compile()
    return nc


def _host_prep(X, edge_weight, Ws, bs, thetas, conv_bs, edge_index):
    """Build per-core device inputs from the raw problem inputs."""
    src = edge_index[0].astype(np.int64)
    dst = edge_index[1].astype(np.int64)
    ew = edge_weight.astype(np.float32)
    deg = np.bincount(src, weights=ew, minlength=N)
    dis = np.where(deg > 0, 1.0 / np.sqrt(np.where(deg > 0, deg, 1.0)), 0.0)
    dis = dis.astype(np.float32)
    w_hat = ((2.0 / LAMBDA_MAX) * (-dis[src] * ew * dis[dst])).astype(np.float32)
    diag = np.float32(2.0 / LAMBDA_MAX - 1.0)
    L = np.zeros((N, N), np.float32)
    np.add.at(L, (dst, src), w_hat)
    if diag != 0.0:
        L[np.arange(N), np.arange(N)] += diag
    L2 = L @ L

    SL = 224.0 / max(float(np.abs(L).max()), 1e-30)
    SL2 = 224.0 / max(float(np.abs(L2).max()), 1e-30)
    ruv_t = np.broadcast_to(
        np.array([[SU / (SH * SL)], [SV / (SH * SL2)]], np.float32),
        (2, 128)).copy()

    # Theta: rows [X; H; U; V] x cols [I|F|T|O].  X/H ride in fp16 scaled
    # by G; U/V ride in the fp8 tensor scaled by ATH (so the G-scaled
    # PSUM contributions match: (SU*U)x(ATH*th) = G*(U*th)).
    Th = np.zeros((4 * F, 4 * F), np.float32)
    Th8 = np.zeros((2 * F, 4 * F), np.float32)
    bias_full = np.zeros(4 * F, np.float32)
    for g in range(4):
        cs = slice(g * F, (g + 1) * F)
        Th[0 * F:1 * F, cs] = Ws[g] * G
        Th[1 * F:2 * F, cs] = (thetas[g, 0] - thetas[g, 2]) * G
        Th8[0 * F:1 * F, cs] = thetas[g, 1] * ATH
        Th8[1 * F:2 * F, cs] = 2.0 * thetas[g, 2] * ATH
        bias_full[cs] = bs[g] + conv_bs[g]
    th_t = np.ascontiguousarray(Th.reshape(GM, 128, 4 * F).astype(np.float16))
    th8_t = np.ascontiguousarray(
        np.clip(Th8, -240.0, 240.0).reshape(4, 128, 4 * F).astype(np_fp8))
    bias_t = np.ascontiguousarray(bias_full.reshape(GM, 128).astype(np.float32))

    in_maps = []
    for i in range(NCORES):
        rows = slice(i * NLOC, (i + 1) * NLOC)
        rhs = np.concatenate([L[rows].T * SL, L2[rows].T * SL2], axis=1)
        rhs = np.clip(rhs, -240.0, 240.0)
        # own-first layout: 5 own tiles + 1 zero pad, then the full
        # 40-tile global block with the own rows zeroed
        own = rhs[rows].reshape(5, 128, NOUT2)
        rest = rhs.copy()
        rest[rows] = 0.0
        ll2 = np.ascontiguousarray(np.concatenate(
            [own, np.zeros((1, 128, NOUT2), np.float32),
             rest.reshape(KT, 128, NOUT2)], axis=0).astype(np_fp8))
        # reference uses Xs = X.reshape(N, T, F) (torch-.view semantics: raw
        # memory reinterpretation), node n's time series is row n of that view
        xi = np.ascontiguousarray(
            X.reshape(N, T, F)[rows].transpose(1, 2, 0)
            .reshape(T, FT, 128, NLOC).astype(np.float16))
        in_maps.append(dict(ll2=ll2, th=th_t, th8=th8_t, xall=xi,
                            biasv=bias_t, ruv=ruv_t))
    return in_maps


def kernel(X, edge_weight, Ws, bs, thetas, conv_bs, edge_index):
    X = np.asarray(X, dtype=np.float32)
    edge_weight = np.asarray(edge_weight, dtype=np.float32)
    Ws = np.asarray(Ws, dtype=np.float32)
    bs = np.asarray(bs, dtype=np.float32)
    thetas = np.asarray(thetas, dtype=np.float32)
    conv_bs = np.asarray(conv_bs, dtype=np.float32)
    edge_index = np.asarray(edge_index)

    in_maps = _host_prep(X, edge_weight, Ws, bs, thetas, conv_bs, edge_index)
    if "nc" not in _CACHE:
        _CACHE["nc"] = _build_nc()
    nc = _CACHE["nc"]
    res = run_bass_kernel_spmd(nc, in_maps, core_ids=list(range(NCORES)))

    H = np.empty((N, F), np.float32)
    C = np.empty((N, F), np.float32)
    for i in range(NCORES):
        rows = slice(i * NLOC, (i + 1) * NLOC)
        H[rows] = res.results[i]["hout"].reshape(F, NLOC).T
        C[rows] = res.results[i]["cout"].reshape(F, NLOC).T
    return H, C



# revision 15
# speedup vs baseline: 1.1162x; 1.0294x over previous
"""GCLSTM (ChebConv-gated LSTM) Trainium2 kernel, 8-core SPMD.

Algorithm notes
---------------
reference computes, per timestep t (T=24) over N=5120 graph nodes:
    gate_g = X_t @ Ws[g] + cheb(H, thetas[g]) + biases      (4 gates)
    cheb(H, th) = H@th0 + (L@H)@th1 + (2L(LH) - H)@th2      (K=3 Chebyshev)
with L the scaled-normalized graph Laplacian (5120x5120, sparse, here
densified).  The Chebyshev basis (U = L@H, V = L^2@H) is shared by all 4
gates, so per step we need exactly ONE dense "mega-prop" [U|V] = [L;L^2]@H
plus the gate matmuls.  Folding:
    gate_g = X_t@Ws[g] + H@(th0-th2) + U@th1 + V@(2*th2) + b
so all gate work is a single [X;H;U;V] (1024) x Theta (1024x1024) matmul.

Sharding: nodes are split across 8 cores (640 each; edges connect
arbitrary nodes, so each core holds the full [L;L^2] column block for its
output rows, resident in SBUF).  The mega-prop contracts over ALL 5120
nodes, so the full H (node-major) is re-assembled every step with two
feature-half AllGathers (~13us each on the critical path); everything
else stays core-local.

Precision: the mega-prop (74% of all PE cycles) runs in fp8-e4m3 with
DoubleRow perf mode (2 fp8 weights per PE cell -> 2 MACs/cycle):
  - [L;L^2] is static: host-quantized to fp8 with global scales SL/SL2.
  - H for the prop is quantized on-device to SH*H fp8 right before the
    AllGather (which therefore also moves half the bytes).  The prop
    PSUM result (SH*SL*U fp32) is rescaled back to exact U at the
    PSUM->SBUF copy, so the gate matmuls see unscaled fp16 U/V.
The gate matmuls stay fp16: X quantization is numerically catastrophic
(X dominates the gate pre-activations; fp8 X alone costs 1.3e-1 rel
err vs 6.8e-3 for the whole prop-fp8 scheme).

Schedule (from NTFF profile analysis): the step's critical chain is
AllGather -> prop -> UV-gates -> LSTM -> transpose/AG-prep.  To keep
the PE fed during the AllGather (and the HAM clock-gate warm), the
contraction is ordered own-nodes-first: ll2 holds [5 own tiles + 1 zero
pad + 40 global tiles with own rows zeroed], and the 3 own pairs per
output half multiply against the locally produced node-major H (hnm8,
no AllGather needed).  DMA queues are split (arrival DMAs on sync,
transposes/agin/x-loads on the scalar HWDGE queue), the fp8 converts
run on gpsimd right before the collectives they feed, and the gate
PSUM->gacc copies/adds run on gpsimd to unload the vector engine,
whose LSTM chain is otherwise on the critical path.
"""
import sys

for _p in ("/opt/trn_rl_repo",):
    if _p not in sys.path:
        sys.path.insert(0, _p)

import numpy as np
import concourse.bass as bass
import concourse.mybir as mybir
import concourse.tile as tile
from concourse import bacc
from concourse.bass_utils import run_bass_kernel_spmd

fp32 = mybir.dt.float32
fp16 = mybir.dt.float16
fp8 = mybir.dt.float8e4
np_fp8 = mybir.dt.np(fp8)

NCORES = 8
B, T, NTOW, F = 512, 24, 10, 256
N = B * NTOW                  # 5120 nodes
NLOC = N // NCORES            # 640 nodes per core
KT = N // 128                 # 40 contraction tiles over nodes
KOWN = 6                      # 5 own tiles + 1 zero pad (even for pairing)
KPO = KOWN // 2               # 3 own DoubleRow pairs
KPF = KT // 2                 # 20 foreign DoubleRow pairs
FT = F // 128                 # 2 feature tiles
GM = (4 * F) // 128           # 8 gate-feature m-tiles
NOUT2 = 2 * NLOC              # 1280 = [U|V] output columns per core
LAMBDA_MAX = 2.0

NCH = [(0, 512), (512, 640)]             # node chunks for gate matmuls
PCH = [(0, 512), (512, 1024), (1024, 1280)]  # [U|V] column chunks

SH = 1.0         # H -> fp8 scale; 1.0 so the fp16->fp8 convert is a pure
                 # cast the SWDGE DMA can do in-flight (power-of-two scaling
                 # doesn't change e4m3 relative precision for |H| < 1)
ATH = 256.0      # theta U/V blocks -> fp8 (|2*th2| <= ~0.22 -> <=56; kept
                 # small so G-scaled gate pre-acts fit fp16 with margin)
SU = 32.0        # U -> fp8 (|U| <~ 3)
SV = 32.0        # V -> fp8 (|V| <~ 6)
G = ATH * SU     # common gate scale: X/H theta blocks (fp16) carry G,
assert G == ATH * SV  # undone for free by the LSTM activation scale

SIG = mybir.ActivationFunctionType.Sigmoid
TANH = mybir.ActivationFunctionType.Tanh
DR = mybir.MatmulPerfMode.DoubleRow

_CACHE = {}


def _build_nc(repeat=1, no_comm=False, split_ag=True):
    nc = bacc.Bacc(None, target_bir_lowering=False, num_devices=NCORES)
    d_ll2 = nc.dram_tensor("ll2", [KOWN + KT, 128, NOUT2], fp8,
                           kind="ExternalInput")
    d_th = nc.dram_tensor("th", [GM, 128, 4 * F], fp16, kind="ExternalInput")
    d_th8 = nc.dram_tensor("th8", [4, 128, 4 * F], fp8, kind="ExternalInput")
    d_x = nc.dram_tensor("xall", [T, FT, 128, NLOC], fp16, kind="ExternalInput")
    d_bias = nc.dram_tensor("biasv", [GM, 128], fp32, kind="ExternalInput")
    d_ruv = nc.dram_tensor("ruv", [2, 128], fp32, kind="ExternalInput")
    d_h = nc.dram_tensor("hout", [FT, 128, NLOC], fp32, kind="ExternalOutput")
    d_c = nc.dram_tensor("cout", [FT, 128, NLOC], fp32, kind="ExternalOutput")

    with tile.TileContext(nc) as tc:
        with (
            tc.tile_pool(name="const", bufs=1) as constp,
            tc.tile_pool(name="xp", bufs=2) as xp,
            tc.tile_pool(name="gp", bufs=2) as gp,
            tc.tile_pool(name="uvp", bufs=1) as uvp,
            tc.tile_pool(name="hp", bufs=2) as hp,
            tc.tile_pool(name="hnmp", bufs=2) as hnmp,
            tc.tile_pool(name="tmpp", bufs=1) as tmpp,
            tc.tile_pool(name="psg", bufs=4, space="PSUM") as psg,
            tc.tile_pool(name="psp", bufs=4, space="PSUM") as psp,
            tc.tile_pool(name="dramio", bufs=2, space="DRAM") as dramp,
        ):
            # ---- resident tensors ----
            sb_ll2 = constp.tile([128, KOWN + KT, NOUT2], fp8, tag="ll2")
            sb_th = constp.tile([128, GM, 4 * F], fp16, tag="th")
            sb_th8 = constp.tile([128, 4, 4 * F], fp8, tag="th8")
            nc.sync.dma_start(sb_th8, d_th8.rearrange("k p j -> p k j"))
            sb_bias = constp.tile([128, GM], fp32, tag="bias")
            sb_ruv = constp.tile([128, 2], fp32, tag="ruv")
            # ft-major so each AllGather-chunk DMA writes contiguous
            # per-partition runs (k*128B descriptors instead of 128B)
            sb_hfull = constp.tile([128, FT, KT, 128], fp8, tag="hfull")
            nc.sync.dma_start(sb_bias, d_bias.rearrange("m p -> p m"))
            nc.sync.dma_start(sb_ruv, d_ruv.rearrange("m p -> p m"))
            # theta in column chunks so step-0 gates can start early
            thv = d_th.rearrange("k p j -> p k j")
            for mc in range(GM):
                cs = slice(mc * 128, (mc + 1) * 128)
                nc.sync.dma_start(sb_th[:, :, cs], thv[:, :, cs])
            x_first = xp.tile([128, FT, NLOC], fp16, tag="x", name="x_first")
            nc.scalar.dma_start(x_first, d_x[0].rearrange("f p n -> p f n"))
            for kg in range((KOWN + KT) // 2):
                ks = slice(kg * 2, (kg + 1) * 2)
                nc.sync.dma_start(
                    sb_ll2[:, ks, :], d_ll2[ks].rearrange("k p j -> p k j"))

            h_fm = None    # current H_i, feature-major [128, FT, NLOC] fp16
            c_fm = None    # current C_i, feature-major fp32
            hnm8_prev = None  # own H slice, node-major fp8 [128, KOWN, F]

            first_iter = True
            gacc = None
            for t in [tt for _r in range(repeat) for tt in range(T)]:
                if first_iter:
                    x_t = x_first
                    first_iter = False
                    gacc = gp.tile([128, GM, NLOC], fp16, tag="g", name="g0")
                    # ---- t=0: X-part inline (H is zero) ----
                    for m in range(GM):
                        pss = [
                            psg.tile([128, c1 - c0], fp32, tag="gps",
                                     name=f"gx0_{m}_{ci}")
                            for ci, (c0, c1) in enumerate(NCH)
                        ]
                        for i, kk in enumerate((0, 1)):
                            for ci, (c0, c1) in enumerate(NCH):
                                nc.tensor.matmul(
                                    pss[ci],
                                    sb_th[:, kk, m * 128:(m + 1) * 128],
                                    x_t[:, kk, c0:c1],
                                    start=(i == 0), stop=(i == 1))
                        for ci, (c0, c1) in enumerate(NCH):
                            if m % 2 == 0:
                                nc.scalar.activation(
                                    gacc[:, m, c0:c1], pss[ci],
                                    mybir.ActivationFunctionType.Copy)
                            else:
                                nc.vector.tensor_copy(gacc[:, m, c0:c1],
                                                      pss[ci])
                else:
                    # ---- gate matmul, H part (fp16; X-part was done
                    # during the previous step's LSTM window) ----
                    for m in range(GM):
                        pss = [
                            psg.tile([128, c1 - c0], fp32, tag="gps",
                                     name=f"gh{t}_{m}_{ci}")
                            for ci, (c0, c1) in enumerate(NCH)
                        ]
                        for i, kk in enumerate((2, 3)):
                            for ci, (c0, c1) in enumerate(NCH):
                                nc.tensor.matmul(
                                    pss[ci],
                                    sb_th[:, kk, m * 128:(m + 1) * 128],
                                    h_fm[:, kk - 2, c0:c1],
                                    start=(i == 0), stop=(i == 1))
                        for ci, (c0, c1) in enumerate(NCH):
                            nc.vector.tensor_add(
                                gacc[:, m, c0:c1], gacc[:, m, c0:c1],
                                pss[ci])

                if t > 0:
                    # ---- mega-prop: fp8 DoubleRow, own pairs first ----
                    u_fm = uvp.tile([128, FT, NLOC], fp8, tag="u", name=f"u{t}")
                    v_fm = uvp.tile([128, FT, NLOC], fp8, tag="v", name=f"v{t}")
                    for m in range(FT):
                        ms = slice(m * 128, (m + 1) * 128)
                        pps = [
                            psp.tile([128, p1 - p0], fp32, tag="pps",
                                     name=f"pps{t}_{m}_{ci}")
                            for ci, (p0, p1) in enumerate(PCH)
                        ]
                        # own pairs first: they only need the locally
                        # produced hnm8 (no AllGather), keeping the PE
                        # busy + HAM warm through the collective latency
                        for k in range(KPO):
                            ksl = slice(2 * k, 2 * k + 2)
                            for ci, (p0, p1) in enumerate(PCH):
                                nc.tensor.matmul(
                                    pps[ci], hnm8_prev[:, ksl, ms],
                                    sb_ll2[:, ksl, p0:p1],
                                    start=(k == 0), stop=False,
                                    perf_mode=DR)
                        if m == 0:
                            # warm-keepers: the PE otherwise idles ~6us
                            # here waiting for the AllGather, HAM gates
                            # the clock to 1.2GHz, and the whole foreign
                            # prop then runs at half rate.  These burn
                            # ~4us of throwaway matmuls to hold K=8/8.
                            for dk in range(14):
                                dps = psg.tile([128, 512], fp32, tag="gps",
                                               name=f"warm{t}_{dk}")
                                nc.tensor.matmul(
                                    dps, sb_th[:, 0, 0:128],
                                    x_t[:, 0, 0:512],
                                    start=True, stop=True)
                        for k in range(KPF):
                            ksl = slice(2 * k, 2 * k + 2)
                            lsl = slice(KOWN + 2 * k, KOWN + 2 * k + 2)
                            for ci, (p0, p1) in enumerate(PCH):
                                nc.tensor.matmul(
                                    pps[ci], sb_hfull[:, m, ksl, :],
                                    sb_ll2[:, lsl, p0:p1],
                                    start=False, stop=(k == KPF - 1),
                                    perf_mode=DR)
                        # rescale copies on scalar: the vector engine's
                        # UV-adds + LSTM chain is the critical tail
                        ru, rv = sb_ruv[:, 0:1], sb_ruv[:, 1:2]
                        CP = mybir.ActivationFunctionType.Copy
                        nc.scalar.activation(u_fm[:, m, 0:512], pps[0], CP,
                                             scale=ru)
                        nc.scalar.activation(u_fm[:, m, 512:640],
                                             pps[1][:, 0:128], CP, scale=ru)
                        nc.scalar.activation(v_fm[:, m, 0:384],
                                             pps[1][:, 128:512], CP, scale=rv)
                        nc.scalar.activation(v_fm[:, m, 384:640], pps[2], CP,
                                             scale=rv)

                    # ---- gate matmul, U/V part (fp8 DoubleRow) ----
                    # even m-tiles first: they feed the ft=0 half of the LSTM,
                    # unblocking the first AllGather half earlier
                    for m in (0, 2, 4, 6, 1, 3, 5, 7):
                        ms = slice(m * 128, (m + 1) * 128)
                        pss = [
                            psg.tile([128, c1 - c0], fp32, tag="gps",
                                     name=f"guv{t}_{m}_{ci}")
                            for ci, (c0, c1) in enumerate(NCH)
                        ]
                        # U over both column chunks, then V: consecutive
                        # matmuls share the stationary operand so its
                        # LDWEIGHTS stays hidden (U,V,U,V exposed ~160ns
                        # of DR weight load per matmul)
                        for ci, (c0, c1) in enumerate(NCH):
                            nc.tensor.matmul(
                                pss[ci], sb_th8[:, 0:2, ms],
                                u_fm[:, 0:2, c0:c1],
                                start=True, stop=False, perf_mode=DR)
                        for ci, (c0, c1) in enumerate(NCH):
                            nc.tensor.matmul(
                                pss[ci], sb_th8[:, 2:4, ms],
                                v_fm[:, 0:2, c0:c1],
                                start=False, stop=True, perf_mode=DR)
                        for ci, (c0, c1) in enumerate(NCH):
                            nc.vector.tensor_add(
                                gacc[:, m, c0:c1], gacc[:, m, c0:c1], pss[ci])

                # ---- LSTM cell (feature-major, elementwise), then transpose
                # the fresh H slice and kick the feature-half AllGathers ----
                last = (t == T - 1)
                h_new = hp.tile([128, FT, NLOC], fp32 if last else fp16,
                                tag="h32" if last else "h", name=f"h{t + 1}",
                                bufs=1 if last else None)
                c_new = hp.tile([128, FT, NLOC], fp32, tag="c", name=f"c{t + 1}")
                if not last:
                    # per-ft transpose targets (contiguous: faster XBAR
                    # transpose + one big descriptor for the cast DMA)
                    hnms = [hnmp.tile([128, 5, 128], fp16, tag=f"hnm{ft}",
                                      name=f"hnm{t}_{ft}") for ft in range(FT)]
                    hnm8 = hnmp.tile([128, KOWN, F], fp8, tag="hnm8",
                                     name=f"hnm8{t}")
                    nc.gpsimd.memset(hnm8[:, 5, :], 0)
                    agins, agouts = [], []
                    nag = FT if split_ag else 1
                    agw = 128 if split_ag else F
                    for ft in range(nag):
                        agins.append(dramp.tile(
                            [NLOC, agw], fp8, tag=f"agin{ft}",
                            name=f"agin{t}_{ft}"))
                        agouts.append(dramp.tile(
                            [N, agw], fp8, tag=f"agout{ft}",
                            addr_space="Shared", name=f"agout{t}_{ft}"))

                def emit_ag(ft):
                    aginv = agins[ft].rearrange("(k p) f -> p k f", p=128)
                    # fp16->fp8 cast happens inside the SWDGE DMA straight
                    # into the collective input (the old tensor_scalar_mul
                    # convert cost 7-10us/step of critical path)
                    nc.gpsimd.dma_start(aginv, hnms[ft])
                    if not no_comm:
                        nc.gpsimd.collective_compute(
                            "AllGather",
                            mybir.AluOpType.bypass,
                            replica_groups=[list(range(NCORES))],
                            ins=[agins[ft].opt()],
                            outs=[agouts[ft].opt()],
                        )
                    agv = agouts[ft].rearrange("(k p) f -> p k f", p=128)
                    # first chunk small so the first foreign prop pair can
                    # start as early as possible after the AllGather lands
                    for k0, k1 in ((0, 2), (2, 6), (6, 16), (16, 28),
                                   (28, 40)):
                        nc.sync.dma_start(sb_hfull[:, ft, k0:k1, :],
                                          agv[:, k0:k1, :])
                # all 8 gate activations first (the ft1 set is then not
                # queued behind ft0's tanh(C) cross-engine wait)
                tis, tfs, tts, tos = [], [], [], []
                for ft in range(FT):
                    ti = tmpp.tile([128, NLOC], fp16, tag=f"t1{ft}",
                                   name=f"ti{t}_{ft}")
                    tf = tmpp.tile([128, NLOC], fp16, tag=f"t2{ft}",
                                   name=f"tf{t}_{ft}")
                    tt = tmpp.tile([128, NLOC], fp16, tag=f"t3{ft}",
                                   name=f"tt{t}_{ft}")
                    to = tmpp.tile([128, NLOC], fp16, tag=f"t4{ft}",
                                   name=f"to{t}_{ft}")
                    nc.scalar.activation(ti, gacc[:, 0 + ft, :], SIG,
                                         bias=sb_bias[:, 0 + ft:1 + ft],
                                         scale=1.0 / G)
                    nc.scalar.activation(tf, gacc[:, 2 + ft, :], SIG,
                                         bias=sb_bias[:, 2 + ft:3 + ft],
                                         scale=1.0 / G)
                    nc.scalar.activation(tt, gacc[:, 4 + ft, :], TANH,
                                         bias=sb_bias[:, 4 + ft:5 + ft],
                                         scale=1.0 / G)
                    nc.scalar.activation(to, gacc[:, 6 + ft, :], SIG,
                                         bias=sb_bias[:, 6 + ft:7 + ft],
                                         scale=1.0 / G)
                    tis.append(ti); tfs.append(tf)
                    tts.append(tt); tos.append(to)
                for ft in range(FT):
                    ti, tf, tt = tis[ft], tfs[ft], tts[ft]
                    if t == 0:
                        nc.vector.tensor_mul(c_new[:, ft, :], ti, tt)
                    else:
                        nc.vector.tensor_mul(ti, ti, tt)
                        nc.vector.tensor_mul(tf, tf, c_fm[:, ft, :])
                        nc.vector.tensor_add(c_new[:, ft, :], ti, tf)
                for ft in range(FT):
                    tc2 = tmpp.tile([128, NLOC], fp16, tag=f"t1{ft}",
                                    name=f"tc{t}_{ft}")
                    nc.scalar.activation(tc2, c_new[:, ft, :], TANH)
                    nc.vector.tensor_mul(h_new[:, ft, :], tos[ft], tc2)
                    if not last:
                        # node-major own slice (feature half ft); sync
                        # HWDGE queue: only the chunk DMAs live there and
                        # they drain ~25us before this point (the scalar
                        # queue stalls behind the LSTM activations)
                        nc.sync.dma_start_transpose(hnms[ft],
                                                    h_new[:, ft, :])
                        if split_ag:
                            emit_ag(ft)
                if not last and not split_ag:
                    emit_ag(0)
                if not last:
                    # own-node fp8 slices for next step's own-first prop
                    # pairs: SBUF->SBUF cast DMAs, emitted after both AG
                    # triggers so they never delay the collectives
                    for ft in range(FT):
                        fs = slice(ft * 128, (ft + 1) * 128)
                        nc.gpsimd.dma_start(hnm8[:, 0:5, fs], hnms[ft])

                # ---- X-part of next step's gates: pure PE filler that
                # runs during the LSTM/AllGather window (needs only the
                # prefetched x_{t+1}), keeping the PE busy and HAM warm
                if t < T - 1 or repeat > 1:
                    tn = t + 1 if t < T - 1 else 0
                    x_n = xp.tile([128, FT, NLOC], fp16, tag="x",
                                  name=f"x{t}_n")
                    nc.scalar.dma_start(x_n,
                                        d_x[tn].rearrange("f p n -> p f n"))
                    gacc_n = gp.tile([128, GM, NLOC], fp16, tag="g",
                                     name=f"g{t}_n")
                    for m in range(GM):
                        pss = [
                            psg.tile([128, c1 - c0], fp32, tag="gps",
                                     name=f"gx{t}n_{m}_{ci}")
                            for ci, (c0, c1) in enumerate(NCH)
                        ]
                        for i, kk in enumerate((0, 1)):
                            for ci, (c0, c1) in enumerate(NCH):
                                nc.tensor.matmul(
                                    pss[ci],
                                    sb_th[:, kk, m * 128:(m + 1) * 128],
                                    x_n[:, kk, c0:c1],
                                    start=(i == 0), stop=(i == 1))
                        for ci, (c0, c1) in enumerate(NCH):
                            if m % 2 == 0:
                                nc.scalar.activation(
                                    gacc_n[:, m, c0:c1], pss[ci],
                                    mybir.ActivationFunctionType.Copy)
                            else:
                                nc.vector.tensor_copy(gacc_n[:, m, c0:c1],
                                                      pss[ci])
                    x_t = x_n
                    gacc = gacc_n
                h_fm, c_fm = h_new, c_new
                if not last:
                    hnm8_prev = hnm8

            nc.sync.dma_start(d_h.rearrange("f p n -> p f n"), h_fm)
            nc.sync.dma_start(d_c.rearrange("f p n -> p f n"), c_fm)

    nc.compile()
    return nc


def _host_prep(X, edge_weight, Ws, bs, thetas, conv_bs, edge_index):
    """Build per-core device inputs from the raw problem inputs."""
    src = edge_index[0].astype(np.int64)
    dst = edge_index[1].astype(np.int64)
    ew = edge_weight.astype(np.float32)
    deg = np.bincount(src, weights=ew, minlength=N)
    dis = np.where(deg > 0, 1.0 / np.sqrt(np.where(deg > 0, deg, 1.0)), 0.0)
    dis = dis.astype(np.float32)
    w_hat = ((2.0 / LAMBDA_MAX) * (-dis[src] * ew * dis[dst])).astype(np.float32)
    diag = np.float32(2.0 / LAMBDA_MAX - 1.0)
    L = np.zeros((N, N), np.float32)
    np.add.at(L, (dst, src), w_hat)
    if diag != 0.0:
        L[np.arange(N), np.arange(N)] += diag
    L2 = L @ L

    SL = 224.0 / max(float(np.abs(L).max()), 1e-30)
    SL2 = 224.0 / max(float(np.abs(L2).max()), 1e-30)
    ruv_t = np.broadcast_to(
        np.array([[SU / (SH * SL)], [SV / (SH * SL2)]], np.float32),
        (2, 128)).copy()

    # Theta: rows [X; H; U; V] x cols [I|F|T|O].  X/H ride in fp16 scaled
    # by G; U/V ride in the fp8 tensor scaled by ATH (so the G-scaled
    # PSUM contributions match: (SU*U)x(ATH*th) = G*(U*th)).
    Th = np.zeros((4 * F, 4 * F), np.float32)
    Th8 = np.zeros((2 * F, 4 * F), np.float32)
    bias_full = np.zeros(4 * F, np.float32)
    for g in range(4):
        cs = slice(g * F, (g + 1) * F)
        Th[0 * F:1 * F, cs] = Ws[g] * G
        Th[1 * F:2 * F, cs] = (thetas[g, 0] - thetas[g, 2]) * G
        Th8[0 * F:1 * F, cs] = thetas[g, 1] * ATH
        Th8[1 * F:2 * F, cs] = 2.0 * thetas[g, 2] * ATH
        bias_full[cs] = bs[g] + conv_bs[g]
    th_t = np.ascontiguousarray(Th.reshape(GM, 128, 4 * F).astype(np.float16))
    th8_t = np.ascontiguousarray(
        np.clip(Th8, -240.0, 240.0).reshape(4, 128, 4 * F).astype(np_fp8))
    bias_t = np.ascontiguousarray(bias_full.reshape(GM, 128).astype(np.float32))

    in_maps = []
    for i in range(NCORES):
        rows = slice(i * NLOC, (i + 1) * NLOC)
        rhs = np.concatenate([L[rows].T * SL, L2[rows].T * SL2], axis=1)
        rhs = np.clip(rhs, -240.0, 240.0)
        # own-first layout: 5 own tiles + 1 zero pad, then the full
        # 40-tile global block with the own rows zeroed
        own = rhs[rows].reshape(5, 128, NOUT2)
        rest = rhs.copy()
        rest[rows] = 0.0
        ll2 = np.ascontiguousarray(np.concatenate(
            [own, np.zeros((1, 128, NOUT2), np.float32),
             rest.reshape(KT, 128, NOUT2)], axis=0).astype(np_fp8))
        # reference uses Xs = X.reshape(N, T, F) (torch-.view semantics: raw
        # memory reinterpretation), node n's time series is row n of that view
        xi = np.ascontiguousarray(
            X.reshape(N, T, F)[rows].transpose(1, 2, 0)
            .reshape(T, FT, 128, NLOC).astype(np.float16))
        in_maps.append(dict(ll2=ll2, th=th_t, th8=th8_t, xall=xi,
                            biasv=bias_t, ruv=ruv_t))
    return in_maps


def kernel(X, edge_weight, Ws, bs, thetas, conv_bs, edge_index):
    X = np.asarray(X, dtype=np.float32)
    edge_weight = np.asarray(edge_weight, dtype=np.float32)
    Ws = np.asarray(Ws, dtype=np.float32)
    bs = np.asarray(bs, dtype=np.float32)
    thetas = np.asarray(thetas, dtype=np.float32)
    conv_bs = np.asarray(conv_bs, dtype=np.float32)
    edge_index = np.asarray(edge_index)

    in_maps = _host_prep(X, edge_weight, Ws, bs, thetas, conv_bs, edge_index)
    if "nc" not in _CACHE:
        _CACHE["nc"] = _build_nc()
    nc = _CACHE["nc"]
    res = run_bass_kernel_spmd(nc, in_maps, core_ids=list(range(NCORES)))

    H = np.empty((N, F), np.float32)
    C = np.empty((N, F), np.float32)
    for i in range(NCORES):
        rows = slice(i * NLOC, (i + 1) * NLOC)
        H[rows] = res.results[i]["hout"].reshape(F, NLOC).T
        C[rows] = res.results[i]["cout"].reshape(F, NLOC).T
    return H, C

